# revision 38
# baseline (speedup 1.0000x reference)
"""Trainium2 Bass kernel for nn_DynamicsShaper: time-varying RBJ lowpass biquad
driven by per-segment-averaged logits.

Sharding: batch row r -> NeuronCore r (8 rows, 8 cores, fully independent).

Per-core layout: the row of T=160000 samples is viewed as [128 partitions x
W=1250].  First-order recurrences (segmented cumsum for run means, reverse
hold-scan for broadcast) use the DVE TensorTensorScan instruction per
partition, chained across partitions via a PE transpose + a [.,128] scan.
The order-2 IIR uses a blocked scan: C=25 chunks of L=50 per partition run
three coupled recursions (zero-state response + two homogeneous solutions)
in lockstep, then chunk-to-chunk affine state maps are combined by a
3-basis walk within each partition and a log2(128)-round Hillis-Steele
(PE shift matrices) across partitions, followed by a linear correction pass.
"""

import sys

sys.path.insert(0, "/opt/trn_rl_repo")

import numpy as np

import concourse.bass as bass
import concourse.bacc as bacc
import concourse.mybir as mybir
import concourse.tile as tile
from concourse import masks

P = 128          # SBUF partitions
W = 1250         # samples per partition (T = P*W)
C = 25           # chunks per partition
L = W // C       # chunk length (50)
T = P * W
B = 8
SR = 16000.0
GAIN_MIN, GAIN_MAX = 0.1, 2.0
LOG_MIN_W = float(np.log(2.0 * np.pi * 20.0 / SR))
LOG_MAX_W = float(np.log(np.pi))
LOG_MIN_Q, LOG_MAX_Q = float(np.log(0.0707)), float(np.log(2.0))

fp = mybir.dt.float32
i32 = mybir.dt.int32
OP = mybir.AluOpType
AF = mybir.ActivationFunctionType


def _act_recip(nc, out, in_, bias=0.0, scale=1.0):
    """ACT-table reciprocal 1/(scale*x + bias); refine with Newton after.
    (bass's activation() helper refuses Reciprocal; build the instruction
    directly -- we always follow with a Newton step on DVE.)"""
    eng = nc.scalar
    inputs = [
        eng.lower_ap(in_),
        mybir.ImmediateValue(dtype=mybir.dt.float32, value=float(bias)),
        mybir.ImmediateValue(dtype=mybir.dt.float32, value=float(scale)),
        mybir.ImmediateValue(dtype=mybir.dt.float32, value=0.0),
    ]
    return eng.add_instruction(
        mybir.InstActivation(
            name=nc.get_next_instruction_name(),
            func=AF.Reciprocal,
            ins=inputs,
            outs=[eng.lower_ap(out)],
        )
    )


DEBUG_TAPS = False


def build_program():
    nc = bacc.Bacc("TRN2", target_bir_lowering=False, debug=False, num_devices=B)
    d_noise = nc.dram_tensor("noise", [P, W], fp, kind="ExternalInput").ap()
    d_seg = nc.dram_tensor("seg", [P, W], i32, kind="ExternalInput").ap()
    d_logits = nc.dram_tensor("logits", [P, 3 * W], fp, kind="ExternalInput").ap()
    d_bnd = nc.dram_tensor("bnd", [P, 2], fp, kind="ExternalInput").ap()
    d_y = nc.dram_tensor("y", [P, W], fp, kind="ExternalOutput").ap()
    taps = {}
    if DEBUG_TAPS:
        def tap(name, ap):
            t = nc.dram_tensor(f"dbg_{name}", list(ap.shape), ap.dtype,
                               kind="ExternalOutput").ap()
            nc.sync.dma_start(t, ap)
            taps[name] = t
    else:
        def tap(name, ap):
            pass
    with tile.TileContext(nc) as tc:
        _body(nc, tc, d_noise, d_seg, d_logits, d_bnd, d_y, tap)
    nc.compile()
    return nc


def _body(nc, tc, d_noise, d_seg, d_logits, d_bnd, d_y, tap=lambda n, a: None):
    from contextlib import ExitStack
    ctx = ExitStack()
    pool = ctx.enter_context(tc.tile_pool(name="main", bufs=1))
    psum = ctx.enter_context(tc.tile_pool(name="ps", bufs=1, space="PSUM"))

    V = nc.vector
    G = nc.gpsimd
    A = nc.scalar

    # ---------- loads ----------
    seg = pool.tile([P, W], i32)
    logits = pool.tile([P, 3 * W], fp)
    noise = pool.tile([P, W], fp)
    cmp = pool.tile([P, W + 1], fp)
    nc.sync.dma_start(cmp[:, 0:1], d_bnd[:, 0:1])
    nc.sync.dma_start(cmp[:, W:W + 1], d_bnd[:, 1:2])
    nc.sync.dma_start(seg[:, 0:W // 2], d_seg[:, 0:W // 2])
    nc.sync.dma_start(seg[:, W // 2:W], d_seg[:, W // 2:W])
    for c in range(3):
        nc.sync.dma_start(logits[:, c * W:(c + 1) * W],
                          d_logits[:, c * W:(c + 1) * W])
    nc.sync.dma_start(noise[:], d_noise)

    # ---------- constants: identity + shift matrices ----------
    ident = pool.tile([P, P], fp)
    masks.make_identity(nc, ident[:])
    ident8 = pool.tile([8, 8], fp)
    masks.make_identity(nc, ident8[:])

    zmat = pool.tile([P, P], fp)
    G.memset(zmat[:], 0.0)

    def shift_mat(base):
        m = pool.tile([P, P], fp, name=f"shift_{base}")
        G.affine_select(out=m[:], in_=zmat[:], compare_op=OP.not_equal, fill=1.0,
                        base=base, pattern=[[-1, P]], channel_multiplier=1)
        return m

    sh_up = {s: shift_mat(s) for s in (1, 2, 4, 8, 16, 32, 64)}  # out[p] = in[p-s]

    # identity-affine pads for HS rounds: rows < s get identity map
    # map layout per 6 cols: (d1, p1, q1, d2, p2, q2); identity: p1=1, q2=1
    idpad = {}
    for s in (1, 2, 4, 8, 16, 32, 64):
        t = pool.tile([P, 6], fp, name=f"idpad_{s}")
        V.memset(t[:], 0.0)
        V.memset(t[0:s, 1:2], 1.0)
        V.memset(t[0:s, 5:6], 1.0)
        idpad[s] = t



    # ---------- gates ----------
    # cmp[P, W+1]: cmp[:, j] (1<=j<=W-1) = (seg[j] == seg[j-1]); col 0 = gate
    # at partition start; col W = "continues into next partition".  The two
    # boundary columns are host-computed (d_bnd) since they need cross-
    # partition neighbors.
    V.tensor_tensor(cmp[:, 1:W], seg[:, 1:], seg[:, :W - 1], OP.is_equal)
    g = cmp[:, 0:W]
    e = cmp[:, 1:W + 1]

    # ---------- forward segmented scans (zero init) ----------
    czero = nc.const_aps.tensor(0.0, (P, W))
    cone = nc.const_aps.tensor(1.0, (P, W))
    d0 = [pool.tile([P, W], fp, name=f"d0_{c}") for c in range(3)]
    l0 = pool.tile([P, W], fp)
    Gp = pool.tile([P, W], fp)   # prefix product of gates (ids sorted!)
    V.tensor_tensor(Gp[:], seg[:], seg[:, 0:1].to_broadcast([P, W]), OP.is_equal)
    V.tensor_scalar_mul(Gp[:], Gp[:], cmp[:, 0:1])
    V.tensor_tensor_scan(l0[:], g, cone, 0.0, OP.mult, OP.add)
    for c in range(3):
        V.tensor_tensor_scan(d0[c][:], g, logits[:, c * W:(c + 1) * W],
                             0.0, OP.mult, OP.add)

    # ---------- cross-partition chain for forward scans ----------
    # summaries [P, 8]: (gP, gP, gP, gP, d0_0[W-1], d0_1[W-1], d0_2[W-1], l0[W-1])
    s8 = pool.tile([P, 8], fp)
    V.tensor_copy(s8[:, 0:4], Gp[:, W - 1:W].to_broadcast([P, 4]))
    for c in range(3):
        V.tensor_copy(s8[:, 4 + c:5 + c], d0[c][:, W - 1:W])
    V.tensor_copy(s8[:, 7:8], l0[:, W - 1:W])
    ps_tg = psum.tile([4, P], fp, tag="ps_a")
    ps_td = psum.tile([4, P], fp, tag="ps_bb")
    nc.tensor.transpose(ps_tg[:], s8[:, 0:4], ident[:])
    nc.tensor.transpose(ps_td[:], s8[:, 4:8], ident[:])
    t8g = pool.tile([4, P], fp)
    t8d = pool.tile([4, P], fp)
    V.tensor_copy(t8g[:], ps_tg[:])
    V.tensor_copy(t8d[:], ps_td[:])
    ch = pool.tile([4, P], fp)
    V.tensor_tensor_scan(ch[:], t8g[:], t8d[:], 0.0, OP.mult, OP.add)
    chs = pool.tile([4, P], fp)   # exclusive: shift right by one, col0 = 0
    V.memset(chs[:, 0:1], 0.0)
    V.tensor_copy(chs[:, 1:P], ch[:, 0:P - 1])
    ps_c = psum.tile([P, 4], fp, tag="ps_cc")
    nc.tensor.matmul(ps_c[:], chs[:], ident8[0:4, 0:4])
    dIn = pool.tile([P, 4], fp)
    V.tensor_copy(dIn[:], ps_c[:])

    # ---------- corrections: d = d0 + G * dIn (write into logits planes) ----------
    d = [logits[:, c * W:(c + 1) * W] for c in range(3)]
    l = pool.tile([P, W], fp)
    V.scalar_tensor_tensor(l[:], Gp[:], dIn[:, 3:4], l0[:], OP.mult, OP.add)
    for c in range(3):
        V.scalar_tensor_tensor(d[c], Gp[:], dIn[:, c:c + 1], d0[c][:],
                               OP.mult, OP.add)

    # ---------- run means at run-ends; reverse hold-scan broadcast ----------
    rl = d0[1]
    _act_recip(nc, rl[:], l[:])  # table recip; error only reaches run means
    ie = pool.tile([P, W], fp)
    A.activation(ie[:], e, AF.Identity, scale=-1.0,
                 bias=nc.const_aps.tensor(1.0, (P, 1)))  # 1-e
    h = l0  # dead after l
    V.tensor_tensor(h[:], ie[:], rl[:], OP.mult)
    dat = [pool.tile([P, W], fp, name=f"dat_{c}") for c in range(3)]
    for c in range(3):
        V.tensor_tensor(dat[c][:], d[c][:], h[:], OP.mult)
    m0 = [pool.tile([P, W], fp, name=f"m0_{c}") for c in range(3)]
    for c in range(3):
        V.tensor_tensor_scan(m0[c][:, ::-1], e[:, ::-1], dat[c][:, ::-1],
                             0.0, OP.mult, OP.add)
    # reverse chain across partitions (descending p)
    Erev = pool.tile([P, W], fp)
    V.tensor_tensor(Erev[:], seg[:], seg[:, W - 1:W].to_broadcast([P, W]), OP.is_equal)
    V.tensor_scalar_mul(Erev[:], Erev[:], cmp[:, W:W + 1])
    s8r = pool.tile([P, 8], fp)
    V.tensor_copy(s8r[:, 0:4], Erev[:, 0:1].to_broadcast([P, 4]))
    for c in range(3):
        V.tensor_copy(s8r[:, 4 + c:5 + c], m0[c][:, 0:1])
    V.memset(s8r[:, 7:8], 0.0)
    ps_t2g = psum.tile([4, P], fp, tag="ps_a")
    ps_t2d = psum.tile([4, P], fp, tag="ps_bb")
    nc.tensor.transpose(ps_t2g[:], s8r[:, 0:4], ident[:])
    nc.tensor.transpose(ps_t2d[:], s8r[:, 4:8], ident[:])
    t8rg = pool.tile([4, P], fp)
    t8rd = pool.tile([4, P], fp)
    V.tensor_copy(t8rg[:], ps_t2g[:])
    V.tensor_copy(t8rd[:], ps_t2d[:])
    chr_ = pool.tile([4, P], fp)
    V.tensor_tensor_scan(chr_[:, ::-1], t8rg[:, ::-1], t8rd[:, ::-1],
                         0.0, OP.mult, OP.add)
    chrs = pool.tile([4, P], fp)  # mIn[p] = chr_[p+1], col W-1... col P-1 = 0
    V.memset(chrs[:, P - 1:P], 0.0)
    V.tensor_copy(chrs[:, 0:P - 1], chr_[:, 1:P])
    ps_c2 = psum.tile([P, 4], fp, tag="ps_cc")
    nc.tensor.matmul(ps_c2[:], chrs[:], ident8[0:4, 0:4])
    mIn = pool.tile([P, 4], fp)
    V.tensor_copy(mIn[:], ps_c2[:])
    # means, broadcast over runs: m = m0 + Erev * mIn (into logits planes)
    m = d  # logits planes; d dead after dat
    for c in (1, 2, 0):
        V.scalar_tensor_tensor(m[c], Erev[:], mIn[:, c:c + 1], m0[c][:],
                               OP.mult, OP.add)

    tap("m0c", m[0])
    tap("m1c", m[1])
    tap("m2c", m[2])
    # ---------- coefficients (ACT chain overlaps DVE FIR work) ----------
    bias_w = pool.tile([P, 1], fp)
    V.memset(bias_w[:], LOG_MIN_W)
    bias_q = pool.tile([P, 1], fp)
    V.memset(bias_q[:], -LOG_MIN_Q)
    bias_hp = pool.tile([P, 1], fp)
    V.memset(bias_hp[:], float(np.pi / 2))
    sg = dat  # reuse: dat tiles dead after the m rescans
    gmin_b = pool.tile([P, 1], fp)
    V.memset(gmin_b[:], GAIN_MIN)
    A.activation(sg[1][:], m[1][:], AF.Sigmoid)
    w = d0[1]  # d0 tiles dead after dat computed
    A.activation(w[:], sg[1][:], AF.Exp, bias=bias_w[:],
                 scale=(LOG_MAX_W - LOG_MIN_W))
    A.activation(sg[2][:], m[2][:], AF.Sigmoid)
    qinv = d0[2]
    A.activation(qinv[:], sg[2][:], AF.Exp, bias=bias_q[:],
                 scale=-(LOG_MAX_Q - LOG_MIN_Q))
    sinw = d0[0]
    A.activation(sinw[:], w[:], AF.Sin)
    alpha = Erev  # dead after m corrections
    V.scalar_tensor_tensor(alpha[:], sinw[:], 0.5, qinv[:], OP.mult, OP.mult)
    r0a = m0[1]  # scratch, dead before this point
    _act_recip(nc, r0a[:], alpha[:], bias=1.0)              # ~1/(1+alpha)
    A.activation(sg[0][:], m[0][:], AF.Sigmoid)
    gain = ie  # h dead after dat
    V.tensor_scalar(gain[:], sg[0][:], GAIN_MAX - GAIN_MIN, GAIN_MIN,
                    OP.mult, OP.add)
    x = m0[0]  # m0 dead after the m corrections
    V.tensor_tensor(x[:], noise[:], gain[:], OP.mult)
    # ---------- FIR accumulate (unscaled): t = x + 2*x[-1] + x[-2] ----------
    ps_x = psum.tile([P, 2], fp, tag="ps_small")
    nc.tensor.matmul(ps_x[:], sh_up[1][:], x[:, W - 2:W])
    xb = pool.tile([P, 2], fp)   # (x[p-1, W-2], x[p-1, W-1]); row0 = 0
    V.tensor_copy(xb[:], ps_x[:])
    s_f = l0  # chain reuse: h -> s_f -> cosw
    V.scalar_tensor_tensor(s_f[:, 2:], x[:, 1:W - 1], 2.0, x[:, 2:], OP.mult, OP.add)
    f = m0[2]
    V.tensor_tensor(f[:, 2:], s_f[:, 2:], x[:, :W - 2], OP.add)
    V.scalar_tensor_tensor(s_f[:, 0:1], xb[:, 1:2], 2.0, x[:, 0:1], OP.mult, OP.add)
    V.tensor_tensor(f[:, 0:1], s_f[:, 0:1], xb[:, 0:1], OP.add)
    V.scalar_tensor_tensor(s_f[:, 1:2], x[:, 0:1], 2.0, x[:, 1:2], OP.mult, OP.add)
    V.tensor_tensor(f[:, 1:2], s_f[:, 1:2], xb[:, 1:2], OP.add)

    # ---------- remaining biquad coefficients ----------
    two_b = pool.tile([P, 1], fp)
    V.memset(two_b[:], 2.0)
    half_b = pool.tile([P, 1], fp)
    V.memset(half_b[:], 0.5)
    nsc2 = d0[2]
    V.scalar_tensor_tensor(nsc2[:], alpha[:], 1.0, r0a[:],
                           OP.add, OP.mult)                  # (1+alpha)*r0
    cosw = l0  # dead after rl
    A.activation(cosw[:], w[:], AF.Sin, bias=bias_hp[:], scale=-1.0)
    nsc3 = sg[1]  # dead after w
    A.activation(nsc3[:], nsc2[:], AF.Identity, scale=-1.0, bias=two_b[:])
    inva0 = cmp  # dead after m corrections
    V.tensor_tensor(inva0[:, 0:W], nsc3[:], r0a[:], OP.mult)
    b0pre = sg[2]  # dead after qinv
    A.activation(b0pre[:], cosw[:], AF.Identity, scale=-0.5, bias=half_b[:])
    b0 = pool.tile([P, W], fp)
    V.tensor_tensor(b0[:], b0pre[:], inva0[:, 0:W], OP.mult)
    na1 = pool.tile([P, W], fp)
    V.scalar_tensor_tensor(na1[:], cosw[:], 2.0, inva0[:, 0:W], OP.mult, OP.mult)
    na2 = pool.tile([P, W], fp)
    V.scalar_tensor_tensor(na2[:], alpha[:], 1.0, inva0[:, 0:W], OP.subtract, OP.mult)
    tap("inva0", inva0[:, 0:W])
    tap("b0", b0[:])
    tap("na1", na1[:])
    tap("na2", na2[:])
    fsc = pool.tile([P, W], fp)
    V.tensor_tensor(fsc[:], f[:], b0[:], OP.mult)
    f = fsc

    # ---------- double-step composite coefficients ----------
    # pair m covers steps n=2m, n=2m+1:
    #   v_n     = na1_n v_{n-1} + na2_n v_{n-2} (+ f_n)
    #   v_{n+1} = A_m  v_{n-1} + B_m  v_{n-2} (+ F_m)
    # with A = na1_{n+1} na1_n + na2_{n+1}, B = na1_{n+1} na2_n,
    #      F = na1_{n+1} f_n + f_{n+1}.
    Lh = L // 2
    na13 = na1.rearrange("p (c n) -> p c n", c=C)
    na23 = na2.rearrange("p (c n) -> p c n", c=C)
    f3 = f.rearrange("p (c n) -> p c n", c=C)
    n1e = na13[:, :, 0:L:2]
    n1o = na13[:, :, 1:L:2]
    n2e = na23[:, :, 0:L:2]
    n2o = na23[:, :, 1:L:2]
    Bm = pool.tile([P, C * Lh], fp)
    Bm3 = Bm.rearrange("p (c m) -> p c m", c=C)
    V.tensor_tensor(Bm3[:], n1o, n2e, OP.mult)
    Amt = pool.tile([P, C * Lh], fp)
    Amt3 = Amt.rearrange("p (c m) -> p c m", c=C)
    V.tensor_tensor(Amt3[:], n1o, n1e, OP.mult)
    Am = pool.tile([P, C * Lh], fp)
    Am3 = Am.rearrange("p (c m) -> p c m", c=C)
    V.tensor_tensor(Am3[:], Amt3[:], n2o, OP.add)
    fD = pool.tile([P, C * Lh * 2], fp)
    fD4 = fD.rearrange("p (c m k) -> p c m k", c=C, m=Lh, k=2)
    V.tensor_tensor(fD4[:, :, :, 1:2], n1o.unsqueeze(3), f3[:, :, 0:L:2].unsqueeze(3),
                    OP.mult)
    V.tensor_tensor(fD4[:, :, :, 1:2], fD4[:, :, :, 1:2], f3[:, :, 1:L:2].unsqueeze(3),
                    OP.add)
    V.tensor_copy(fD4[:, :, :, 0:1], f3[:, :, 0:L:2].unsqueeze(3))
    coefD = pool.tile([P, C * Lh * 12], fp)
    cD4 = coefD.rearrange("p (c m k) -> p c m k", c=C, m=Lh, k=12)
    A.activation(cD4[:, :, :, 0:3], n2e.unsqueeze(3).to_broadcast([P, C, Lh, 3]),
                 AF.Copy)
    A.activation(cD4[:, :, :, 3:6], n1e.unsqueeze(3).to_broadcast([P, C, Lh, 3]),
                 AF.Copy)
    V.tensor_copy(cD4[:, :, :, 6:9], Bm3.unsqueeze(3).to_broadcast([P, C, Lh, 3]))
    V.tensor_copy(cD4[:, :, :, 9:12], Am3.unsqueeze(3).to_broadcast([P, C, Lh, 3]))

    # ---------- within-chunk recursions (y_zs, p, q interleaved) ----------
    # ypq[P, C, (L+2)*3]: slot k holds 3 values (y, p, q) for recursion index
    # k-2; slots 0,1 are the initial conditions.
    ypq = pool.tile([P, C * (L + 2) * 3], fp)
    ypq3 = ypq.rearrange("p (c m) -> p c m", c=C)
    V.memset(ypq3[:, :, 0:6], 0.0)
    V.memset(ypq3[:, :, 2:3], 1.0)   # q_{-2} = 1
    V.memset(ypq3[:, :, 4:5], 1.0)   # p_{-1} = 1
    u = pool.tile([P, C * 12], fp)
    u4 = u.rearrange("p (c s k) -> p c s k", c=C, s=2, k=6)
    for m in range(Lh):
        n = 2 * m
        prevs = ypq3[:, :, 3 * n:3 * n + 6].unsqueeze(2).to_broadcast(
            [P, C, 2, 6])
        coefv = cD4[:, :, m, :].rearrange("p c (s k) -> p c s k", s=2, k=6)
        V.tensor_tensor(u4[:], prevs, coefv, OP.mult)
        V.tensor_tensor(
            ypq3[:, :, 3 * n + 6:3 * n + 12].rearrange(
                "p c (s k) -> p c s k", s=2, k=3),
            u4[:, :, :, 0:3], u4[:, :, :, 3:6], OP.add)
        V.tensor_tensor(ypq3[:, :, 3 * n + 6:3 * n + 10:3],
                        ypq3[:, :, 3 * n + 6:3 * n + 10:3],
                        fD4[:, :, m, :], OP.add)

    tap("f", f[:])
    tap("coefD", coefD[:])
    tap("fD", fD[:])
    tap("ypq", ypq[:])
    # ---------- pair-composed chunk maps + 3-basis walk ----------
    # Pair k combines chunks (2k, 2k+1); the leftover chunk C-1 is applied as
    # a final single step.  Pair-map layout: (d2, p2, q2, d1, p1, q1).
    NPAIR = C // 2
    NSTEP = NPAIR + 1
    base = 3 * L
    arow1 = ypq3[:, 0:2 * NPAIR:2, base + 3:base + 6]   # (d1,p1,q1) of evens
    arow2 = ypq3[:, 0:2 * NPAIR:2, base:base + 3]       # (d2,p2,q2) of evens
    mapsP = pool.tile([P, NPAIR * 6], fp)
    mp3 = mapsP.rearrange("p (k m) -> p k m", k=NPAIR)
    vA = pool.tile([P, NPAIR * 3], fp)
    vB = pool.tile([P, NPAIR * 3], fp)
    vC = pool.tile([P, NPAIR * 3], fp)
    v3a = vA.rearrange("p (k m) -> p k m", k=NPAIR)
    v3b = vB.rearrange("p (k m) -> p k m", k=NPAIR)
    v3c = vC.rearrange("p (k m) -> p k m", k=NPAIR)

    def bsc(col):
        return ypq3[:, 1:2 * NPAIR + 1:2, base + col:base + col + 1]

    for (pc, qc, dc), off in (((4, 5, 3), 3), ((1, 2, 0), 0)):
        V.tensor_tensor(v3a[:], arow1, bsc(pc).to_broadcast([P, NPAIR, 3]),
                        OP.mult)
        V.tensor_tensor(v3b[:], arow2, bsc(qc).to_broadcast([P, NPAIR, 3]),
                        OP.mult)
        V.tensor_tensor(v3c[:], v3a[:], v3b[:], OP.add)
        V.tensor_tensor(mp3[:, :, off:off + 1], v3c[:, :, 0:1], bsc(dc), OP.add)
        V.tensor_copy(mp3[:, :, off + 1:off + 3], v3c[:, :, 1:3])

    # walk: slot j holds incoming state of chunk 2j (j < NSTEP); the final
    # slot NSTEP is the partition's outgoing state.
    # state slot pair order: (beta, alpha) = (y_{-2}, y_{-1}); walks: 0 = zero
    # state, 1 = alpha basis, 2 = beta basis.
    S = pool.tile([P, 3 * (NSTEP + 1) * 2], fp)
    S4 = S.rearrange("p (w s k) -> p w s k", w=3, s=NSTEP + 1, k=2)
    V.memset(S[:], 0.0)
    V.memset(S4[:, 1:2, 0:1, 1:2], 1.0)
    V.memset(S4[:, 2:3, 0:1, 0:1], 1.0)
    wk = pool.tile([P, 12], fp)
    wk4 = wk.rearrange("p (w r s) -> p w r s", w=3, r=2, s=2)
    wkb = pool.tile([P, 6], fp)
    wkb3 = wkb.rearrange("p (w r) -> p w r", w=3, r=2)
    for j in range(NSTEP):
        if j < NPAIR:
            bv2 = mp3[:, j, :].rearrange("p (a b) -> p a b", a=2, b=3)
        else:
            c = 2 * NPAIR
            bv2 = ypq3[:, c, base:base + 6].rearrange("p (a b) -> p a b",
                                                      a=2, b=3)
        W4 = bv2[:, :, 1:3].unsqueeze(1).to_broadcast([P, 3, 2, 2])
        dpv = bv2[:, :, 0:1].unsqueeze(1).to_broadcast([P, 3, 2, 1]).rearrange(
            "p w r s -> p w (r s)")
        # (alpha, beta) repeated per row: stored order is (beta, alpha)
        X = S4[:, :, j:j + 1, ::-1].rearrange(
            "p w s k -> p w (s k)").unsqueeze(2).to_broadcast([P, 3, 2, 2])
        V.tensor_tensor(wk4[:], W4, X, OP.mult)
        V.tensor_tensor(wkb3[:], wk4[:, :, :, 0:1].rearrange(
            "p w r s -> p w (r s)"), wk4[:, :, :, 1:2].rearrange(
            "p w r s -> p w (r s)"), OP.add)
        V.tensor_tensor(S4[:, :, j + 1, :], wkb3[:], dpv, OP.add)

    # ---------- partition-level affine maps ----------
    # Mcur[P, 6] = (d1, p1, q1, d2, p2, q2):  alpha' = p1 a + q1 b + d1 etc.
    Mcur = pool.tile([P, 6], fp)
    Snap = S4[:, :, NSTEP:NSTEP + 1, :]  # [P, 3, 1, 2]
    for row, comp in ((0, 1), (1, 0)):  # row 0: alpha (k=1), row 1: beta (k=0)
        sv = Snap[:, :, :, comp:comp + 1].rearrange("p a b c -> p (a b c)")
        dsc = Snap[:, 0:1, :, comp:comp + 1].rearrange(
            "p a b c -> p (a b c)").to_broadcast([P, 3])
        V.tensor_tensor(Mcur[:, 3 * row:3 * row + 3], sv, dsc, OP.subtract)
        V.tensor_copy(Mcur[:, 3 * row:3 * row + 1],
                      Snap[:, 0:1, :, comp:comp + 1].rearrange(
                          "p a b c -> p (a b c)"))

    # ---------- Hillis-Steele inclusive scan of affine maps over partitions ----
    Mnew = pool.tile([P, 6], fp)
    ash = pool.tile([P, 6], fp)
    v6 = pool.tile([P, 6], fp)
    u1t = pool.tile([P, 6], fp)
    u2t = pool.tile([P, 6], fp)
    ps_m = psum.tile([P, 6], fp)
    idmap = pool.tile([P, 6], fp)
    V.memset(idmap[:], 0.0)
    V.memset(idmap[:, 1:2], 1.0)
    V.memset(idmap[:, 5:6], 1.0)
    cur, new = Mcur, Mnew
    for s in (1, 2, 4, 8, 16, 32, 64):
        nc.tensor.matmul(ps_m[:], sh_up[s][:], cur[:])
        V.tensor_tensor(ash[:], ps_m[:], idpad[s][:], OP.add)
        a2 = ash.rearrange("p (r k) -> p r k", r=2)       # a rows
        bp = cur[:, 1:5:3].rearrange("p r -> p r").unsqueeze(2).to_broadcast(
            [P, 2, 3])                                     # (bp1, bp2)
        bq = cur[:, 2:6:3].unsqueeze(2).to_broadcast([P, 2, 3])
        bd = cur[:, 0:4:3].unsqueeze(2)                    # [P, 2, 1]
        a1g = a2[:, 0:1, :].to_broadcast([P, 2, 3])        # (ad1, ap1, aq1)
        a2g = a2[:, 1:2, :].to_broadcast([P, 2, 3])        # (ad2, ap2, aq2)
        u1 = u1t.rearrange("p (r k) -> p r k", r=2)
        u2 = u2t.rearrange("p (r k) -> p r k", r=2)
        v = v6.rearrange("p (r k) -> p r k", r=2)
        nw = new.rearrange("p (r k) -> p r k", r=2)
        V.tensor_tensor(u1[:], a1g, bp, OP.mult)
        V.tensor_tensor(u2[:], a2g, bq, OP.mult)
        V.tensor_tensor(v[:], u1[:], u2[:], OP.add)
        V.tensor_tensor(nw[:, :, 0:1], v[:, :, 0:1], bd, OP.add)
        V.tensor_copy(nw[:, :, 1:3], v[:, :, 1:3])
        cur, new = new, cur
    # exclusive d: alpha0/beta0 per partition = d-cols of T^hat_{p-1}
    ps_d = psum.tile([P, 2], fp, tag="ps_small")
    nc.tensor.matmul(ps_d[:], sh_up[1][:], cur[:, 0:4:3])
    ab0 = pool.tile([P, 2], fp)   # (alpha0, beta0)
    V.tensor_copy(ab0[:], ps_d[:])

    # ---------- true per-chunk incoming states ----------
    # even slots: s_true = s_w0 + alpha0*(s_w1-s_w0) + beta0*(s_w2-s_w0);
    # odd chunks: apply the even chunk's own map to the true even state.
    dl2 = pool.tile([P, 2 * NSTEP], fp)
    dl3 = pool.tile([P, 2 * NSTEP], fp)
    tre = pool.tile([P, 2 * NSTEP], fp)  # [alpha_true 0:NSTEP | beta_true]
    for comp, off in ((1, 0), (0, NSTEP)):
        s0 = S4[:, 0:1, 0:NSTEP, comp:comp + 1].rearrange("p a b c -> p (a b c)")
        s1 = S4[:, 1:2, 0:NSTEP, comp:comp + 1].rearrange("p a b c -> p (a b c)")
        s2 = S4[:, 2:3, 0:NSTEP, comp:comp + 1].rearrange("p a b c -> p (a b c)")
        V.tensor_tensor(dl2[:, off:off + NSTEP], s1, s0, OP.subtract)
        V.tensor_tensor(dl3[:, off:off + NSTEP], s2, s0, OP.subtract)
        V.scalar_tensor_tensor(tre[:, off:off + NSTEP], dl2[:, off:off + NSTEP],
                               ab0[:, 0:1], s0, OP.mult, OP.add)
        V.scalar_tensor_tensor(tre[:, off:off + NSTEP], dl3[:, off:off + NSTEP],
                               ab0[:, 1:2], tre[:, off:off + NSTEP],
                               OP.mult, OP.add)
    ale = tre[:, 0:NPAIR]
    bee = tre[:, NSTEP:NSTEP + NPAIR]

    def ev(col):
        return ypq3[:, 0:2 * NPAIR:2, base + col:base + col + 1].rearrange(
            "p c k -> p (c k)")

    od1 = vA[:, 0:NPAIR]
    od2 = vB[:, 0:NPAIR]
    alo = vA[:, NPAIR:2 * NPAIR]
    beo = vB[:, NPAIR:2 * NPAIR]
    V.tensor_tensor(od1[:], ev(4), ale, OP.mult)
    V.tensor_tensor(od2[:], ev(5), bee, OP.mult)
    V.tensor_tensor(od1[:], od1[:], od2[:], OP.add)
    V.tensor_tensor(alo[:], od1[:], ev(3), OP.add)
    V.tensor_tensor(od1[:], ev(1), ale, OP.mult)
    V.tensor_tensor(od2[:], ev(2), bee, OP.mult)
    V.tensor_tensor(od1[:], od1[:], od2[:], OP.add)
    V.tensor_tensor(beo[:], od1[:], ev(0), OP.add)
    atr = pool.tile([P, 2 * C], fp)   # cols: [alpha_true (C), beta_true (C)]
    V.tensor_copy(atr[:, 0:C:2], tre[:, 0:NSTEP])
    V.tensor_copy(atr[:, 1:C:2], alo[:])
    V.tensor_copy(atr[:, C:2 * C:2], tre[:, NSTEP:2 * NSTEP])
    V.tensor_copy(atr[:, C + 1:2 * C:2], beo[:])

    # ---------- correction pass: y = y_zs + p*alpha_c + q*beta_c ----------
    yfin = pool.tile([P, W], fp)
    y3 = yfin.rearrange("p (c n) -> p c n", c=C)
    t1 = pool.tile([P, W], fp)
    t13 = t1.rearrange("p (c n) -> p c n", c=C)
    t2 = pool.tile([P, W], fp)
    t23 = t2.rearrange("p (c n) -> p c n", c=C)
    alc = atr[:, 0:C].unsqueeze(2).to_broadcast([P, C, L])
    bec = atr[:, C:2 * C].unsqueeze(2).to_broadcast([P, C, L])
    Ch = C // 2
    for lo, hi in ((0, Ch), (Ch, C)):
        pv = ypq3[:, lo:hi, 7:6 + 3 * L:3]
        qv = ypq3[:, lo:hi, 8:6 + 3 * L:3]
        yzs = ypq3[:, lo:hi, 6:4 + 3 * L:3]
        V.tensor_tensor(t13[:, lo:hi, :], pv, alc[:, lo:hi, :], OP.mult)
        V.tensor_tensor(t23[:, lo:hi, :], qv, bec[:, lo:hi, :], OP.mult)
        V.tensor_tensor(y3[:, lo:hi, :], t13[:, lo:hi, :], yzs, OP.add)
        V.tensor_tensor(y3[:, lo:hi, :], y3[:, lo:hi, :], t23[:, lo:hi, :],
                        OP.add)
        nc.sync.dma_start(d_y[:, lo * L:hi * L], yfin[:, lo * L:hi * L])
    tap("atr", atr[:])


_NC_CACHE = None


def _get_nc():
    global _NC_CACHE
    if _NC_CACHE is None:
        _NC_CACHE = build_program()
    return _NC_CACHE


def make_in_maps(noise, seg, lg):
    maps = []
    for r in range(len(noise)):
        s2 = seg[r].reshape(P, W)
        bnd = np.zeros((P, 2), np.float32)
        bnd[1:, 0] = (s2[1:, 0] == s2[:-1, W - 1])
        bnd[:-1, 1] = (s2[1:, 0] == s2[:-1, W - 1])
        maps.append({
            "noise": noise[r].reshape(P, W),
            "seg": s2,
            "logits": np.concatenate(
                [lg[r, :, c].reshape(P, W) for c in range(3)], axis=1),
            "bnd": bnd,
        })
    return maps


def kernel(noise_bursts, segment_ids, logits):
    from concourse.bass_utils import run_bass_kernel_spmd

    noise = np.ascontiguousarray(np.asarray(noise_bursts, dtype=np.float32))
    seg = np.ascontiguousarray(np.asarray(segment_ids).astype(np.int32))
    lg = np.ascontiguousarray(np.asarray(logits, dtype=np.float32))
    assert noise.shape == (B, T) and seg.shape == (B, T) and lg.shape == (B, T, 3)

    nc = _get_nc()
    in_maps = make_in_maps(noise, seg, lg)
    res = run_bass_kernel_spmd(nc, in_maps, list(range(B)))
    out = np.stack([res.results[r]["y"].reshape(T) for r in range(B)])
    return out.astype(np.float32)


# revision 39
# speedup vs baseline: 1.0100x; 1.0100x over previous
"""Trainium2 Bass kernel for nn_DynamicsShaper: time-varying RBJ lowpass biquad
driven by per-segment-averaged logits.

Sharding: batch row r -> NeuronCore r (8 rows, 8 cores, fully independent).

Per-core layout: the row of T=160000 samples is viewed as [128 partitions x
W=1250].  First-order recurrences (segmented cumsum for run means, reverse
hold-scan for broadcast) use the DVE TensorTensorScan instruction per
partition, chained across partitions via a PE transpose + a [.,128] scan.
The order-2 IIR uses a blocked scan: C=25 chunks of L=50 per partition run
three coupled recursions (zero-state response + two homogeneous solutions)
in lockstep, then chunk-to-chunk affine state maps are combined by a
3-basis walk within each partition and a log2(128)-round Hillis-Steele
(PE shift matrices) across partitions, followed by a linear correction pass.
"""

import sys

sys.path.insert(0, "/opt/trn_rl_repo")

import numpy as np

import concourse.bass as bass
import concourse.bacc as bacc
import concourse.mybir as mybir
import concourse.tile as tile
from concourse import masks

P = 128          # SBUF partitions
W = 1250         # samples per partition (T = P*W)
C = 25           # chunks per partition
L = W // C       # chunk length (50)
T = P * W
B = 8
SR = 16000.0
GAIN_MIN, GAIN_MAX = 0.1, 2.0
LOG_MIN_W = float(np.log(2.0 * np.pi * 20.0 / SR))
LOG_MAX_W = float(np.log(np.pi))
LOG_MIN_Q, LOG_MAX_Q = float(np.log(0.0707)), float(np.log(2.0))

fp = mybir.dt.float32
i32 = mybir.dt.int32
OP = mybir.AluOpType
AF = mybir.ActivationFunctionType


def _act_recip(nc, out, in_, bias=0.0, scale=1.0):
    """ACT-table reciprocal 1/(scale*x + bias); refine with Newton after.
    (bass's activation() helper refuses Reciprocal; build the instruction
    directly -- we always follow with a Newton step on DVE.)"""
    eng = nc.scalar
    inputs = [
        eng.lower_ap(in_),
        mybir.ImmediateValue(dtype=mybir.dt.float32, value=float(bias)),
        mybir.ImmediateValue(dtype=mybir.dt.float32, value=float(scale)),
        mybir.ImmediateValue(dtype=mybir.dt.float32, value=0.0),
    ]
    return eng.add_instruction(
        mybir.InstActivation(
            name=nc.get_next_instruction_name(),
            func=AF.Reciprocal,
            ins=inputs,
            outs=[eng.lower_ap(out)],
        )
    )


DEBUG_TAPS = False


def build_program():
    nc = bacc.Bacc("TRN2", target_bir_lowering=False, debug=False, num_devices=B)
    d_noise = nc.dram_tensor("noise", [P, W], fp, kind="ExternalInput").ap()
    d_seg = nc.dram_tensor("seg", [P, W], i32, kind="ExternalInput").ap()
    d_logits = nc.dram_tensor("logits", [P, 3 * W], fp, kind="ExternalInput").ap()
    d_bnd = nc.dram_tensor("bnd", [P, 2], fp, kind="ExternalInput").ap()
    d_y = nc.dram_tensor("y", [P, W], fp, kind="ExternalOutput").ap()
    taps = {}
    if DEBUG_TAPS:
        def tap(name, ap):
            t = nc.dram_tensor(f"dbg_{name}", list(ap.shape), ap.dtype,
                               kind="ExternalOutput").ap()
            nc.sync.dma_start(t, ap)
            taps[name] = t
    else:
        def tap(name, ap):
            pass
    with tile.TileContext(nc) as tc:
        _body(nc, tc, d_noise, d_seg, d_logits, d_bnd, d_y, tap)
    nc.compile()
    return nc


def _body(nc, tc, d_noise, d_seg, d_logits, d_bnd, d_y, tap=lambda n, a: None):
    from contextlib import ExitStack
    ctx = ExitStack()
    pool = ctx.enter_context(tc.tile_pool(name="main", bufs=1))
    psum = ctx.enter_context(tc.tile_pool(name="ps", bufs=1, space="PSUM"))

    V = nc.vector
    G = nc.gpsimd
    A = nc.scalar

    # ---------- loads ----------
    seg = pool.tile([P, W], i32)
    logits = pool.tile([P, 3 * W], fp)
    noise = pool.tile([P, W], fp)
    cmp = pool.tile([P, W + 1], fp)
    nc.sync.dma_start(cmp[:, 0:1], d_bnd[:, 0:1])
    nc.sync.dma_start(cmp[:, W:W + 1], d_bnd[:, 1:2])
    nc.sync.dma_start(seg[:, 0:W // 2], d_seg[:, 0:W // 2])
    nc.sync.dma_start(seg[:, W // 2:W], d_seg[:, W // 2:W])
    for c in range(3):
        nc.sync.dma_start(logits[:, c * W:(c + 1) * W],
                          d_logits[:, c * W:(c + 1) * W])
    nc.sync.dma_start(noise[:], d_noise)

    # ---------- constants: identity + shift matrices ----------
    ident = pool.tile([P, P], fp)
    masks.make_identity(nc, ident[:])
    ident8 = pool.tile([8, 8], fp)
    masks.make_identity(nc, ident8[:])

    zmat = pool.tile([P, P], fp)
    G.memset(zmat[:], 0.0)

    def shift_mat(base):
        m = pool.tile([P, P], fp, name=f"shift_{base}")
        G.affine_select(out=m[:], in_=zmat[:], compare_op=OP.not_equal, fill=1.0,
                        base=base, pattern=[[-1, P]], channel_multiplier=1)
        return m

    sh_up = {s: shift_mat(s) for s in (1, 2, 4, 8, 16, 32, 64)}  # out[p] = in[p-s]

    # identity-affine pads for HS rounds: rows < s get identity map
    # map layout per 6 cols: (d1, p1, q1, d2, p2, q2); identity: p1=1, q2=1
    idpad = {}
    for s in (1, 2, 4, 8, 16, 32, 64):
        t = pool.tile([P, 6], fp, name=f"idpad_{s}")
        V.memset(t[:], 0.0)
        V.memset(t[0:s, 1:2], 1.0)
        V.memset(t[0:s, 5:6], 1.0)
        idpad[s] = t



    # ---------- gates ----------
    # cmp[P, W+1]: cmp[:, j] (1<=j<=W-1) = (seg[j] == seg[j-1]); col 0 = gate
    # at partition start; col W = "continues into next partition".  The two
    # boundary columns are host-computed (d_bnd) since they need cross-
    # partition neighbors.
    V.tensor_tensor(cmp[:, 1:W], seg[:, 1:], seg[:, :W - 1], OP.is_equal)
    g = cmp[:, 0:W]
    e = cmp[:, 1:W + 1]

    # ---------- forward segmented scans (zero init) ----------
    czero = nc.const_aps.tensor(0.0, (P, W))
    cone = nc.const_aps.tensor(1.0, (P, W))
    d0 = [pool.tile([P, W], fp, name=f"d0_{c}") for c in range(3)]
    l0 = pool.tile([P, W], fp)
    Gp = pool.tile([P, W], fp)   # prefix product of gates (ids sorted!)
    V.tensor_tensor(Gp[:], seg[:], seg[:, 0:1].to_broadcast([P, W]), OP.is_equal)
    V.tensor_scalar_mul(Gp[:], Gp[:], cmp[:, 0:1])
    V.tensor_tensor_scan(l0[:], g, cone, 0.0, OP.mult, OP.add)
    for c in range(3):
        V.tensor_tensor_scan(d0[c][:], g, logits[:, c * W:(c + 1) * W],
                             0.0, OP.mult, OP.add)

    # ---------- cross-partition chain for forward scans ----------
    # summaries [P, 8]: (gP, gP, gP, gP, d0_0[W-1], d0_1[W-1], d0_2[W-1], l0[W-1])
    s8 = pool.tile([P, 8], fp)
    V.tensor_copy(s8[:, 0:4], Gp[:, W - 1:W].to_broadcast([P, 4]))
    for c in range(3):
        V.tensor_copy(s8[:, 4 + c:5 + c], d0[c][:, W - 1:W])
    V.tensor_copy(s8[:, 7:8], l0[:, W - 1:W])
    ps_tg = psum.tile([4, P], fp, tag="ps_a")
    ps_td = psum.tile([4, P], fp, tag="ps_bb")
    nc.tensor.transpose(ps_tg[:], s8[:, 0:4], ident[:])
    nc.tensor.transpose(ps_td[:], s8[:, 4:8], ident[:])
    t8g = pool.tile([4, P], fp)
    t8d = pool.tile([4, P], fp)
    V.tensor_copy(t8g[:], ps_tg[:])
    V.tensor_copy(t8d[:], ps_td[:])
    ch = pool.tile([4, P], fp)
    V.tensor_tensor_scan(ch[:], t8g[:], t8d[:], 0.0, OP.mult, OP.add)
    chs = pool.tile([4, P], fp)   # exclusive: shift right by one, col0 = 0
    V.memset(chs[:, 0:1], 0.0)
    V.tensor_copy(chs[:, 1:P], ch[:, 0:P - 1])
    ps_c = psum.tile([P, 4], fp, tag="ps_cc")
    nc.tensor.matmul(ps_c[:], chs[:], ident8[0:4, 0:4])
    dIn = pool.tile([P, 4], fp)
    V.tensor_copy(dIn[:], ps_c[:])

    # ---------- corrections: d = d0 + G * dIn (write into logits planes) ----------
    d = [logits[:, c * W:(c + 1) * W] for c in range(3)]
    l = pool.tile([P, W], fp)
    V.scalar_tensor_tensor(l[:], Gp[:], dIn[:, 3:4], l0[:], OP.mult, OP.add)
    for c in range(3):
        V.scalar_tensor_tensor(d[c], Gp[:], dIn[:, c:c + 1], d0[c][:],
                               OP.mult, OP.add)

    # ---------- run means at run-ends; reverse hold-scan broadcast ----------
    rl = d0[1]
    _act_recip(nc, rl[:], l[:])  # table recip; error only reaches run means
    ie = pool.tile([P, W], fp)
    A.activation(ie[:], e, AF.Identity, scale=-1.0,
                 bias=nc.const_aps.tensor(1.0, (P, 1)))  # 1-e
    h = l0  # dead after l
    V.tensor_tensor(h[:], ie[:], rl[:], OP.mult)
    dat = [pool.tile([P, W], fp, name=f"dat_{c}") for c in range(3)]
    for c in range(3):
        V.tensor_tensor(dat[c][:], d[c][:], h[:], OP.mult)
    m0 = [pool.tile([P, W], fp, name=f"m0_{c}") for c in range(3)]
    for c in range(3):
        V.tensor_tensor_scan(m0[c][:, ::-1], e[:, ::-1], dat[c][:, ::-1],
                             0.0, OP.mult, OP.add)
    # reverse chain across partitions (descending p)
    Erev = pool.tile([P, W], fp)
    V.tensor_tensor(Erev[:], seg[:], seg[:, W - 1:W].to_broadcast([P, W]), OP.is_equal)
    V.tensor_scalar_mul(Erev[:], Erev[:], cmp[:, W:W + 1])
    s8r = pool.tile([P, 8], fp)
    V.tensor_copy(s8r[:, 0:4], Erev[:, 0:1].to_broadcast([P, 4]))
    for c in range(3):
        V.tensor_copy(s8r[:, 4 + c:5 + c], m0[c][:, 0:1])
    V.memset(s8r[:, 7:8], 0.0)
    ps_t2g = psum.tile([4, P], fp, tag="ps_a")
    ps_t2d = psum.tile([4, P], fp, tag="ps_bb")
    nc.tensor.transpose(ps_t2g[:], s8r[:, 0:4], ident[:])
    nc.tensor.transpose(ps_t2d[:], s8r[:, 4:8], ident[:])
    t8rg = pool.tile([4, P], fp)
    t8rd = pool.tile([4, P], fp)
    V.tensor_copy(t8rg[:], ps_t2g[:])
    V.tensor_copy(t8rd[:], ps_t2d[:])
    chr_ = pool.tile([4, P], fp)
    V.tensor_tensor_scan(chr_[:, ::-1], t8rg[:, ::-1], t8rd[:, ::-1],
                         0.0, OP.mult, OP.add)
    chrs = pool.tile([4, P], fp)  # mIn[p] = chr_[p+1], col W-1... col P-1 = 0
    V.memset(chrs[:, P - 1:P], 0.0)
    V.tensor_copy(chrs[:, 0:P - 1], chr_[:, 1:P])
    ps_c2 = psum.tile([P, 4], fp, tag="ps_cc")
    nc.tensor.matmul(ps_c2[:], chrs[:], ident8[0:4, 0:4])
    mIn = pool.tile([P, 4], fp)
    V.tensor_copy(mIn[:], ps_c2[:])
    # means, broadcast over runs: m = m0 + Erev * mIn (into logits planes)
    m = d  # logits planes; d dead after dat
    for c in (0, 1, 2):
        V.scalar_tensor_tensor(m[c], Erev[:], mIn[:, c:c + 1], m0[c][:],
                               OP.mult, OP.add)

    tap("m0c", m[0])
    tap("m1c", m[1])
    tap("m2c", m[2])
    # ---------- coefficients (ACT chain overlaps DVE FIR work) ----------
    bias_w = pool.tile([P, 1], fp)
    V.memset(bias_w[:], LOG_MIN_W)
    bias_q = pool.tile([P, 1], fp)
    V.memset(bias_q[:], -LOG_MIN_Q)
    bias_hp = pool.tile([P, 1], fp)
    V.memset(bias_hp[:], float(np.pi / 2))
    sg = dat  # reuse: dat tiles dead after the m rescans
    gmin_b = pool.tile([P, 1], fp)
    V.memset(gmin_b[:], GAIN_MIN)
    A.activation(sg[0][:], m[0][:], AF.Sigmoid)
    gain = ie  # h dead after dat
    V.tensor_scalar(gain[:], sg[0][:], GAIN_MAX - GAIN_MIN, GAIN_MIN,
                    OP.mult, OP.add)
    A.activation(sg[1][:], m[1][:], AF.Sigmoid)
    w = d0[1]  # d0 tiles dead after dat computed
    A.activation(w[:], sg[1][:], AF.Exp, bias=bias_w[:],
                 scale=(LOG_MAX_W - LOG_MIN_W))
    A.activation(sg[2][:], m[2][:], AF.Sigmoid)
    qinv = d0[2]
    A.activation(qinv[:], sg[2][:], AF.Exp, bias=bias_q[:],
                 scale=-(LOG_MAX_Q - LOG_MIN_Q))
    sinw = d0[0]
    A.activation(sinw[:], w[:], AF.Sin)
    x = m0[0]  # m0 dead after the m corrections
    V.tensor_tensor(x[:], noise[:], gain[:], OP.mult)
    # ---------- FIR accumulate (unscaled): t = x + 2*x[-1] + x[-2] ----------
    ps_x = psum.tile([P, 2], fp, tag="ps_small")
    nc.tensor.matmul(ps_x[:], sh_up[1][:], x[:, W - 2:W])
    xb = pool.tile([P, 2], fp)   # (x[p-1, W-2], x[p-1, W-1]); row0 = 0
    V.tensor_copy(xb[:], ps_x[:])
    s_f = m0[1]
    V.scalar_tensor_tensor(s_f[:, 2:], x[:, 1:W - 1], 2.0, x[:, 2:], OP.mult, OP.add)
    f = m0[2]
    V.tensor_tensor(f[:, 2:], s_f[:, 2:], x[:, :W - 2], OP.add)
    V.scalar_tensor_tensor(s_f[:, 0:1], xb[:, 1:2], 2.0, x[:, 0:1], OP.mult, OP.add)
    V.tensor_tensor(f[:, 0:1], s_f[:, 0:1], xb[:, 0:1], OP.add)
    V.scalar_tensor_tensor(s_f[:, 1:2], x[:, 0:1], 2.0, x[:, 1:2], OP.mult, OP.add)
    V.tensor_tensor(f[:, 1:2], s_f[:, 1:2], xb[:, 1:2], OP.add)

    # ---------- remaining biquad coefficients ----------
    alpha = Erev  # dead after m corrections
    V.scalar_tensor_tensor(alpha[:], sinw[:], 0.5, qinv[:], OP.mult, OP.mult)
    r0a = m0[1]  # s_f scratch, dead once the FIR sums are built
    _act_recip(nc, r0a[:], alpha[:], bias=1.0)              # ~1/(1+alpha)
    two_b = pool.tile([P, 1], fp)
    V.memset(two_b[:], 2.0)
    half_b = pool.tile([P, 1], fp)
    V.memset(half_b[:], 0.5)
    nsc2 = d0[2]
    V.scalar_tensor_tensor(nsc2[:], alpha[:], 1.0, r0a[:],
                           OP.add, OP.mult)                  # (1+alpha)*r0
    cosw = l0  # dead after rl
    A.activation(cosw[:], w[:], AF.Sin, bias=bias_hp[:], scale=-1.0)
    nsc3 = sg[1]  # dead after w
    A.activation(nsc3[:], nsc2[:], AF.Identity, scale=-1.0, bias=two_b[:])
    inva0 = cmp  # dead after m corrections
    V.tensor_tensor(inva0[:, 0:W], nsc3[:], r0a[:], OP.mult)
    b0pre = sg[2]  # dead after qinv
    A.activation(b0pre[:], cosw[:], AF.Identity, scale=-0.5, bias=half_b[:])
    b0 = pool.tile([P, W], fp)
    V.tensor_tensor(b0[:], b0pre[:], inva0[:, 0:W], OP.mult)
    na1 = pool.tile([P, W], fp)
    V.scalar_tensor_tensor(na1[:], cosw[:], 2.0, inva0[:, 0:W], OP.mult, OP.mult)
    na2 = pool.tile([P, W], fp)
    V.scalar_tensor_tensor(na2[:], alpha[:], 1.0, inva0[:, 0:W], OP.subtract, OP.mult)
    tap("inva0", inva0[:, 0:W])
    tap("b0", b0[:])
    tap("na1", na1[:])
    tap("na2", na2[:])
    fsc = pool.tile([P, W], fp)
    V.tensor_tensor(fsc[:], f[:], b0[:], OP.mult)
    f = fsc

    # ---------- double-step composite coefficients ----------
    # pair m covers steps n=2m, n=2m+1:
    #   v_n     = na1_n v_{n-1} + na2_n v_{n-2} (+ f_n)
    #   v_{n+1} = A_m  v_{n-1} + B_m  v_{n-2} (+ F_m)
    # with A = na1_{n+1} na1_n + na2_{n+1}, B = na1_{n+1} na2_n,
    #      F = na1_{n+1} f_n + f_{n+1}.
    Lh = L // 2
    na13 = na1.rearrange("p (c n) -> p c n", c=C)
    na23 = na2.rearrange("p (c n) -> p c n", c=C)
    f3 = f.rearrange("p (c n) -> p c n", c=C)
    n1e = na13[:, :, 0:L:2]
    n1o = na13[:, :, 1:L:2]
    n2e = na23[:, :, 0:L:2]
    n2o = na23[:, :, 1:L:2]
    Bm = pool.tile([P, C * Lh], fp)
    Bm3 = Bm.rearrange("p (c m) -> p c m", c=C)
    V.tensor_tensor(Bm3[:], n1o, n2e, OP.mult)
    Amt = pool.tile([P, C * Lh], fp)
    Amt3 = Amt.rearrange("p (c m) -> p c m", c=C)
    V.tensor_tensor(Amt3[:], n1o, n1e, OP.mult)
    Am = pool.tile([P, C * Lh], fp)
    Am3 = Am.rearrange("p (c m) -> p c m", c=C)
    V.tensor_tensor(Am3[:], Amt3[:], n2o, OP.add)
    fD = pool.tile([P, C * Lh * 2], fp)
    fD4 = fD.rearrange("p (c m k) -> p c m k", c=C, m=Lh, k=2)
    V.tensor_tensor(fD4[:, :, :, 1:2], n1o.unsqueeze(3), f3[:, :, 0:L:2].unsqueeze(3),
                    OP.mult)
    V.tensor_tensor(fD4[:, :, :, 1:2], fD4[:, :, :, 1:2], f3[:, :, 1:L:2].unsqueeze(3),
                    OP.add)
    V.tensor_copy(fD4[:, :, :, 0:1], f3[:, :, 0:L:2].unsqueeze(3))
    coefD = pool.tile([P, C * Lh * 12], fp)
    cD4 = coefD.rearrange("p (c m k) -> p c m k", c=C, m=Lh, k=12)
    A.activation(cD4[:, :, :, 0:3], n2e.unsqueeze(3).to_broadcast([P, C, Lh, 3]),
                 AF.Copy)
    A.activation(cD4[:, :, :, 3:6], n1e.unsqueeze(3).to_broadcast([P, C, Lh, 3]),
                 AF.Copy)
    V.tensor_copy(cD4[:, :, :, 6:9], Bm3.unsqueeze(3).to_broadcast([P, C, Lh, 3]))
    V.tensor_copy(cD4[:, :, :, 9:12], Am3.unsqueeze(3).to_broadcast([P, C, Lh, 3]))

    # ---------- within-chunk recursions (y_zs, p, q interleaved) ----------
    # ypq[P, C, (L+2)*3]: slot k holds 3 values (y, p, q) for recursion index
    # k-2; slots 0,1 are the initial conditions.
    ypq = pool.tile([P, C * (L + 2) * 3], fp)
    ypq3 = ypq.rearrange("p (c m) -> p c m", c=C)
    V.memset(ypq3[:, :, 0:6], 0.0)
    V.memset(ypq3[:, :, 2:3], 1.0)   # q_{-2} = 1
    V.memset(ypq3[:, :, 4:5], 1.0)   # p_{-1} = 1
    u = pool.tile([P, C * 12], fp)
    u4 = u.rearrange("p (c s k) -> p c s k", c=C, s=2, k=6)
    for m in range(Lh):
        n = 2 * m
        prevs = ypq3[:, :, 3 * n:3 * n + 6].unsqueeze(2).to_broadcast(
            [P, C, 2, 6])
        coefv = cD4[:, :, m, :].rearrange("p c (s k) -> p c s k", s=2, k=6)
        V.tensor_tensor(u4[:], prevs, coefv, OP.mult)
        V.tensor_tensor(
            ypq3[:, :, 3 * n + 6:3 * n + 12].rearrange(
                "p c (s k) -> p c s k", s=2, k=3),
            u4[:, :, :, 0:3], u4[:, :, :, 3:6], OP.add)
        V.tensor_tensor(ypq3[:, :, 3 * n + 6:3 * n + 10:3],
                        ypq3[:, :, 3 * n + 6:3 * n + 10:3],
                        fD4[:, :, m, :], OP.add)

    tap("f", f[:])
    tap("coefD", coefD[:])
    tap("fD", fD[:])
    tap("ypq", ypq[:])
    # ---------- pair-composed chunk maps + 3-basis walk ----------
    # Pair k combines chunks (2k, 2k+1); the leftover chunk C-1 is applied as
    # a final single step.  Pair-map layout: (d2, p2, q2, d1, p1, q1).
    NPAIR = C // 2
    NSTEP = NPAIR + 1
    base = 3 * L
    arow1 = ypq3[:, 0:2 * NPAIR:2, base + 3:base + 6]   # (d1,p1,q1) of evens
    arow2 = ypq3[:, 0:2 * NPAIR:2, base:base + 3]       # (d2,p2,q2) of evens
    mapsP = pool.tile([P, NPAIR * 6], fp)
    mp3 = mapsP.rearrange("p (k m) -> p k m", k=NPAIR)
    vA = pool.tile([P, NPAIR * 3], fp)
    vB = pool.tile([P, NPAIR * 3], fp)
    vC = pool.tile([P, NPAIR * 3], fp)
    v3a = vA.rearrange("p (k m) -> p k m", k=NPAIR)
    v3b = vB.rearrange("p (k m) -> p k m", k=NPAIR)
    v3c = vC.rearrange("p (k m) -> p k m", k=NPAIR)

    def bsc(col):
        return ypq3[:, 1:2 * NPAIR + 1:2, base + col:base + col + 1]

    for (pc, qc, dc), off in (((4, 5, 3), 3), ((1, 2, 0), 0)):
        V.tensor_tensor(v3a[:], arow1, bsc(pc).to_broadcast([P, NPAIR, 3]),
                        OP.mult)
        V.tensor_tensor(v3b[:], arow2, bsc(qc).to_broadcast([P, NPAIR, 3]),
                        OP.mult)
        V.tensor_tensor(v3c[:], v3a[:], v3b[:], OP.add)
        V.tensor_tensor(mp3[:, :, off:off + 1], v3c[:, :, 0:1], bsc(dc), OP.add)
        V.tensor_copy(mp3[:, :, off + 1:off + 3], v3c[:, :, 1:3])

    # walk: slot j holds incoming state of chunk 2j (j < NSTEP); the final
    # slot NSTEP is the partition's outgoing state.
    # state slot pair order: (beta, alpha) = (y_{-2}, y_{-1}); walks: 0 = zero
    # state, 1 = alpha basis, 2 = beta basis.
    S = pool.tile([P, 3 * (NSTEP + 1) * 2], fp)
    S4 = S.rearrange("p (w s k) -> p w s k", w=3, s=NSTEP + 1, k=2)
    V.memset(S[:], 0.0)
    V.memset(S4[:, 1:2, 0:1, 1:2], 1.0)
    V.memset(S4[:, 2:3, 0:1, 0:1], 1.0)
    wk = pool.tile([P, 12], fp)
    wk4 = wk.rearrange("p (w r s) -> p w r s", w=3, r=2, s=2)
    wkb = pool.tile([P, 6], fp)
    wkb3 = wkb.rearrange("p (w r) -> p w r", w=3, r=2)
    for j in range(NSTEP):
        if j < NPAIR:
            bv2 = mp3[:, j, :].rearrange("p (a b) -> p a b", a=2, b=3)
        else:
            c = 2 * NPAIR
            bv2 = ypq3[:, c, base:base + 6].rearrange("p (a b) -> p a b",
                                                      a=2, b=3)
        W4 = bv2[:, :, 1:3].unsqueeze(1).to_broadcast([P, 3, 2, 2])
        dpv = bv2[:, :, 0:1].unsqueeze(1).to_broadcast([P, 3, 2, 1]).rearrange(
            "p w r s -> p w (r s)")
        # (alpha, beta) repeated per row: stored order is (beta, alpha)
        X = S4[:, :, j:j + 1, ::-1].rearrange(
            "p w s k -> p w (s k)").unsqueeze(2).to_broadcast([P, 3, 2, 2])
        V.tensor_tensor(wk4[:], W4, X, OP.mult)
        V.tensor_tensor(wkb3[:], wk4[:, :, :, 0:1].rearrange(
            "p w r s -> p w (r s)"), wk4[:, :, :, 1:2].rearrange(
            "p w r s -> p w (r s)"), OP.add)
        V.tensor_tensor(S4[:, :, j + 1, :], wkb3[:], dpv, OP.add)

    # ---------- partition-level affine maps ----------
    # Mcur[P, 6] = (d1, p1, q1, d2, p2, q2):  alpha' = p1 a + q1 b + d1 etc.
    Mcur = pool.tile([P, 6], fp)
    Snap = S4[:, :, NSTEP:NSTEP + 1, :]  # [P, 3, 1, 2]
    for row, comp in ((0, 1), (1, 0)):  # row 0: alpha (k=1), row 1: beta (k=0)
        sv = Snap[:, :, :, comp:comp + 1].rearrange("p a b c -> p (a b c)")
        dsc = Snap[:, 0:1, :, comp:comp + 1].rearrange(
            "p a b c -> p (a b c)").to_broadcast([P, 3])
        V.tensor_tensor(Mcur[:, 3 * row:3 * row + 3], sv, dsc, OP.subtract)
        V.tensor_copy(Mcur[:, 3 * row:3 * row + 1],
                      Snap[:, 0:1, :, comp:comp + 1].rearrange(
                          "p a b c -> p (a b c)"))

    # ---------- Hillis-Steele inclusive scan of affine maps over partitions ----
    Mnew = pool.tile([P, 6], fp)
    ash = pool.tile([P, 6], fp)
    v6 = pool.tile([P, 6], fp)
    u1t = pool.tile([P, 6], fp)
    u2t = pool.tile([P, 6], fp)
    ps_m = psum.tile([P, 6], fp)
    idmap = pool.tile([P, 6], fp)
    V.memset(idmap[:], 0.0)
    V.memset(idmap[:, 1:2], 1.0)
    V.memset(idmap[:, 5:6], 1.0)
    cur, new = Mcur, Mnew
    for s in (1, 2, 4, 8, 16, 32, 64):
        nc.tensor.matmul(ps_m[:], sh_up[s][:], cur[:])
        V.tensor_tensor(ash[:], ps_m[:], idpad[s][:], OP.add)
        a2 = ash.rearrange("p (r k) -> p r k", r=2)       # a rows
        bp = cur[:, 1:5:3].rearrange("p r -> p r").unsqueeze(2).to_broadcast(
            [P, 2, 3])                                     # (bp1, bp2)
        bq = cur[:, 2:6:3].unsqueeze(2).to_broadcast([P, 2, 3])
        bd = cur[:, 0:4:3].unsqueeze(2)                    # [P, 2, 1]
        a1g = a2[:, 0:1, :].to_broadcast([P, 2, 3])        # (ad1, ap1, aq1)
        a2g = a2[:, 1:2, :].to_broadcast([P, 2, 3])        # (ad2, ap2, aq2)
        u1 = u1t.rearrange("p (r k) -> p r k", r=2)
        u2 = u2t.rearrange("p (r k) -> p r k", r=2)
        v = v6.rearrange("p (r k) -> p r k", r=2)
        nw = new.rearrange("p (r k) -> p r k", r=2)
        V.tensor_tensor(u1[:], a1g, bp, OP.mult)
        V.tensor_tensor(u2[:], a2g, bq, OP.mult)
        V.tensor_tensor(v[:], u1[:], u2[:], OP.add)
        V.tensor_tensor(nw[:, :, 0:1], v[:, :, 0:1], bd, OP.add)
        V.tensor_copy(nw[:, :, 1:3], v[:, :, 1:3])
        cur, new = new, cur
    # exclusive d: alpha0/beta0 per partition = d-cols of T^hat_{p-1}
    ps_d = psum.tile([P, 2], fp, tag="ps_small")
    nc.tensor.matmul(ps_d[:], sh_up[1][:], cur[:, 0:4:3])
    ab0 = pool.tile([P, 2], fp)   # (alpha0, beta0)
    V.tensor_copy(ab0[:], ps_d[:])

    # ---------- true per-chunk incoming states ----------
    # even slots: s_true = s_w0 + alpha0*(s_w1-s_w0) + beta0*(s_w2-s_w0);
    # odd chunks: apply the even chunk's own map to the true even state.
    dl2 = pool.tile([P, 2 * NSTEP], fp)
    dl3 = pool.tile([P, 2 * NSTEP], fp)
    tre = pool.tile([P, 2 * NSTEP], fp)  # [alpha_true 0:NSTEP | beta_true]
    for comp, off in ((1, 0), (0, NSTEP)):
        s0 = S4[:, 0:1, 0:NSTEP, comp:comp + 1].rearrange("p a b c -> p (a b c)")
        s1 = S4[:, 1:2, 0:NSTEP, comp:comp + 1].rearrange("p a b c -> p (a b c)")
        s2 = S4[:, 2:3, 0:NSTEP, comp:comp + 1].rearrange("p a b c -> p (a b c)")
        V.tensor_tensor(dl2[:, off:off + NSTEP], s1, s0, OP.subtract)
        V.tensor_tensor(dl3[:, off:off + NSTEP], s2, s0, OP.subtract)
        V.scalar_tensor_tensor(tre[:, off:off + NSTEP], dl2[:, off:off + NSTEP],
                               ab0[:, 0:1], s0, OP.mult, OP.add)
        V.scalar_tensor_tensor(tre[:, off:off + NSTEP], dl3[:, off:off + NSTEP],
                               ab0[:, 1:2], tre[:, off:off + NSTEP],
                               OP.mult, OP.add)
    ale = tre[:, 0:NPAIR]
    bee = tre[:, NSTEP:NSTEP + NPAIR]

    def ev(col):
        return ypq3[:, 0:2 * NPAIR:2, base + col:base + col + 1].rearrange(
            "p c k -> p (c k)")

    od1 = vA[:, 0:NPAIR]
    od2 = vB[:, 0:NPAIR]
    alo = vA[:, NPAIR:2 * NPAIR]
    beo = vB[:, NPAIR:2 * NPAIR]
    V.tensor_tensor(od1[:], ev(4), ale, OP.mult)
    V.tensor_tensor(od2[:], ev(5), bee, OP.mult)
    V.tensor_tensor(od1[:], od1[:], od2[:], OP.add)
    V.tensor_tensor(alo[:], od1[:], ev(3), OP.add)
    V.tensor_tensor(od1[:], ev(1), ale, OP.mult)
    V.tensor_tensor(od2[:], ev(2), bee, OP.mult)
    V.tensor_tensor(od1[:], od1[:], od2[:], OP.add)
    V.tensor_tensor(beo[:], od1[:], ev(0), OP.add)
    atr = pool.tile([P, 2 * C], fp)   # cols: [alpha_true (C), beta_true (C)]
    V.tensor_copy(atr[:, 0:C:2], tre[:, 0:NSTEP])
    V.tensor_copy(atr[:, 1:C:2], alo[:])
    V.tensor_copy(atr[:, C:2 * C:2], tre[:, NSTEP:2 * NSTEP])
    V.tensor_copy(atr[:, C + 1:2 * C:2], beo[:])

    # ---------- correction pass: y = y_zs + p*alpha_c + q*beta_c ----------
    yfin = pool.tile([P, W], fp)
    y3 = yfin.rearrange("p (c n) -> p c n", c=C)
    t1 = pool.tile([P, W], fp)
    t13 = t1.rearrange("p (c n) -> p c n", c=C)
    t2 = pool.tile([P, W], fp)
    t23 = t2.rearrange("p (c n) -> p c n", c=C)
    alc = atr[:, 0:C].unsqueeze(2).to_broadcast([P, C, L])
    bec = atr[:, C:2 * C].unsqueeze(2).to_broadcast([P, C, L])
    Ch = C // 2
    for lo, hi in ((0, Ch), (Ch, C)):
        pv = ypq3[:, lo:hi, 7:6 + 3 * L:3]
        qv = ypq3[:, lo:hi, 8:6 + 3 * L:3]
        yzs = ypq3[:, lo:hi, 6:4 + 3 * L:3]
        V.tensor_tensor(t13[:, lo:hi, :], pv, alc[:, lo:hi, :], OP.mult)
        V.tensor_tensor(t23[:, lo:hi, :], qv, bec[:, lo:hi, :], OP.mult)
        V.tensor_tensor(y3[:, lo:hi, :], t13[:, lo:hi, :], yzs, OP.add)
        V.tensor_tensor(y3[:, lo:hi, :], y3[:, lo:hi, :], t23[:, lo:hi, :],
                        OP.add)
        nc.sync.dma_start(d_y[:, lo * L:hi * L], yfin[:, lo * L:hi * L])
    tap("atr", atr[:])


_NC_CACHE = None


def _get_nc():
    global _NC_CACHE
    if _NC_CACHE is None:
        _NC_CACHE = build_program()
    return _NC_CACHE


def make_in_maps(noise, seg, lg):
    maps = []
    for r in range(len(noise)):
        s2 = seg[r].reshape(P, W)
        bnd = np.zeros((P, 2), np.float32)
        bnd[1:, 0] = (s2[1:, 0] == s2[:-1, W - 1])
        bnd[:-1, 1] = (s2[1:, 0] == s2[:-1, W - 1])
        maps.append({
            "noise": noise[r].reshape(P, W),
            "seg": s2,
            "logits": np.concatenate(
                [lg[r, :, c].reshape(P, W) for c in range(3)], axis=1),
            "bnd": bnd,
        })
    return maps


def kernel(noise_bursts, segment_ids, logits):
    from concourse.bass_utils import run_bass_kernel_spmd

    noise = np.ascontiguousarray(np.asarray(noise_bursts, dtype=np.float32))
    seg = np.ascontiguousarray(np.asarray(segment_ids).astype(np.int32))
    lg = np.ascontiguousarray(np.asarray(logits, dtype=np.float32))
    assert noise.shape == (B, T) and seg.shape == (B, T) and lg.shape == (B, T, 3)

    nc = _get_nc()
    in_maps = make_in_maps(noise, seg, lg)
    res = run_bass_kernel_spmd(nc, in_maps, list(range(B)))
    out = np.stack([res.results[r]["y"].reshape(T) for r in range(B)])
    return out.astype(np.float32)


# revision 41
# speedup vs baseline: 1.0179x; 1.0078x over previous
"""Trainium2 Bass kernel for nn_DynamicsShaper: time-varying RBJ lowpass biquad
driven by per-segment-averaged logits.

Sharding: batch row r -> NeuronCore r (8 rows, 8 cores, fully independent).

Per-core layout: the row of T=160000 samples is viewed as [128 partitions x
W=1250].  First-order recurrences (segmented cumsum for run means, reverse
hold-scan for broadcast) use the DVE TensorTensorScan instruction per
partition, chained across partitions via a PE transpose + a [.,128] scan.
The order-2 IIR uses a blocked scan: C=25 chunks of L=50 per partition run
three coupled recursions (zero-state response + two homogeneous solutions)
in lockstep, then chunk-to-chunk affine state maps are combined by a
3-basis walk within each partition and a log2(128)-round Hillis-Steele
(PE shift matrices) across partitions, followed by a linear correction pass.
"""

import sys

sys.path.insert(0, "/opt/trn_rl_repo")

import numpy as np

import concourse.bass as bass
import concourse.bacc as bacc
import concourse.mybir as mybir
import concourse.tile as tile
from concourse import masks

P = 128          # SBUF partitions
W = 1250         # samples per partition (T = P*W)
C = 25           # chunks per partition
L = W // C       # chunk length (50)
T = P * W
B = 8
SR = 16000.0
GAIN_MIN, GAIN_MAX = 0.1, 2.0
LOG_MIN_W = float(np.log(2.0 * np.pi * 20.0 / SR))
LOG_MAX_W = float(np.log(np.pi))
LOG_MIN_Q, LOG_MAX_Q = float(np.log(0.0707)), float(np.log(2.0))

fp = mybir.dt.float32
i32 = mybir.dt.int32
OP = mybir.AluOpType
AF = mybir.ActivationFunctionType


def _act_recip(nc, out, in_, bias=0.0, scale=1.0):
    """ACT-table reciprocal 1/(scale*x + bias); refine with Newton after.
    (bass's activation() helper refuses Reciprocal; build the instruction
    directly -- we always follow with a Newton step on DVE.)"""
    eng = nc.scalar
    inputs = [
        eng.lower_ap(in_),
        mybir.ImmediateValue(dtype=mybir.dt.float32, value=float(bias)),
        mybir.ImmediateValue(dtype=mybir.dt.float32, value=float(scale)),
        mybir.ImmediateValue(dtype=mybir.dt.float32, value=0.0),
    ]
    return eng.add_instruction(
        mybir.InstActivation(
            name=nc.get_next_instruction_name(),
            func=AF.Reciprocal,
            ins=inputs,
            outs=[eng.lower_ap(out)],
        )
    )


DEBUG_TAPS = False


def build_program():
    nc = bacc.Bacc("TRN2", target_bir_lowering=False, debug=False, num_devices=B)
    d_noise = nc.dram_tensor("noise", [P, W], fp, kind="ExternalInput").ap()
    d_seg = nc.dram_tensor("seg", [P, W], i32, kind="ExternalInput").ap()
    d_logits = nc.dram_tensor("logits", [P, 3 * W], fp, kind="ExternalInput").ap()
    d_bnd = nc.dram_tensor("bnd", [P, 2], fp, kind="ExternalInput").ap()
    d_y = nc.dram_tensor("y", [P, W], fp, kind="ExternalOutput").ap()
    taps = {}
    if DEBUG_TAPS:
        def tap(name, ap):
            t = nc.dram_tensor(f"dbg_{name}", list(ap.shape), ap.dtype,
                               kind="ExternalOutput").ap()
            nc.sync.dma_start(t, ap)
            taps[name] = t
    else:
        def tap(name, ap):
            pass
    with tile.TileContext(nc) as tc:
        _body(nc, tc, d_noise, d_seg, d_logits, d_bnd, d_y, tap)
    nc.compile()
    return nc


def _body(nc, tc, d_noise, d_seg, d_logits, d_bnd, d_y, tap=lambda n, a: None):
    from contextlib import ExitStack
    ctx = ExitStack()
    pool = ctx.enter_context(tc.tile_pool(name="main", bufs=1))
    psum = ctx.enter_context(tc.tile_pool(name="ps", bufs=1, space="PSUM"))

    V = nc.vector
    G = nc.gpsimd
    A = nc.scalar

    # ---------- loads ----------
    seg = pool.tile([P, W], i32)
    logits = pool.tile([P, 3 * W], fp)
    noise = pool.tile([P, W], fp)
    cmp = pool.tile([P, W + 1], fp)
    nc.sync.dma_start(cmp[:, 0:1], d_bnd[:, 0:1])
    nc.sync.dma_start(cmp[:, W:W + 1], d_bnd[:, 1:2])
    nc.sync.dma_start(seg[:, 0:W // 2], d_seg[:, 0:W // 2])
    nc.sync.dma_start(seg[:, W // 2:W], d_seg[:, W // 2:W])
    for c in range(3):
        nc.sync.dma_start(logits[:, c * W:(c + 1) * W],
                          d_logits[:, c * W:(c + 1) * W])
    nc.sync.dma_start(noise[:], d_noise)

    # ---------- constants: identity + shift matrices ----------
    ident = pool.tile([P, P], fp)
    masks.make_identity(nc, ident[:])
    ident8 = pool.tile([8, 8], fp)
    masks.make_identity(nc, ident8[:])

    zmat = pool.tile([P, P], fp)
    G.memset(zmat[:], 0.0)

    def shift_mat(base):
        m = pool.tile([P, P], fp, name=f"shift_{base}")
        G.affine_select(out=m[:], in_=zmat[:], compare_op=OP.not_equal, fill=1.0,
                        base=base, pattern=[[-1, P]], channel_multiplier=1)
        return m

    sh_up = {s: shift_mat(s) for s in (1, 2, 4, 8, 16, 32, 64)}  # out[p] = in[p-s]

    # identity-affine pads for HS rounds: rows < s get identity map
    # map layout per 6 cols: (d1, p1, q1, d2, p2, q2); identity: p1=1, q2=1
    idpad = {}
    for s in (1, 2, 4, 8, 16, 32, 64):
        t = pool.tile([P, 6], fp, name=f"idpad_{s}")
        V.memset(t[:], 0.0)
        V.memset(t[0:s, 1:2], 1.0)
        V.memset(t[0:s, 5:6], 1.0)
        idpad[s] = t



    # ---------- gates ----------
    # cmp[P, W+1]: cmp[:, j] (1<=j<=W-1) = (seg[j] == seg[j-1]); col 0 = gate
    # at partition start; col W = "continues into next partition".  The two
    # boundary columns are host-computed (d_bnd) since they need cross-
    # partition neighbors.
    V.tensor_tensor(cmp[:, 1:W], seg[:, 1:], seg[:, :W - 1], OP.is_equal)
    g = cmp[:, 0:W]
    e = cmp[:, 1:W + 1]

    # ---------- forward segmented scans (zero init) ----------
    czero = nc.const_aps.tensor(0.0, (P, W))
    cone = nc.const_aps.tensor(1.0, (P, W))
    d0 = [pool.tile([P, W], fp, name=f"d0_{c}") for c in range(3)]
    l0 = pool.tile([P, W], fp)
    Gp = pool.tile([P, W], fp)   # prefix product of gates (ids sorted!)
    V.tensor_tensor(Gp[:], seg[:], seg[:, 0:1].to_broadcast([P, W]), OP.is_equal)
    V.tensor_scalar_mul(Gp[:], Gp[:], cmp[:, 0:1])
    V.tensor_tensor_scan(l0[:], g, cone, 0.0, OP.mult, OP.add)
    for c in range(3):
        V.tensor_tensor_scan(d0[c][:], g, logits[:, c * W:(c + 1) * W],
                             0.0, OP.mult, OP.add)

    # ---------- cross-partition chain for forward scans ----------
    # summaries [P, 8]: (gP, gP, gP, gP, d0_0[W-1], d0_1[W-1], d0_2[W-1], l0[W-1])
    s8 = pool.tile([P, 8], fp)
    V.tensor_copy(s8[:, 0:4], Gp[:, W - 1:W].to_broadcast([P, 4]))
    for c in range(3):
        V.tensor_copy(s8[:, 4 + c:5 + c], d0[c][:, W - 1:W])
    V.tensor_copy(s8[:, 7:8], l0[:, W - 1:W])
    ps_tg = psum.tile([4, P], fp, tag="ps_a")
    ps_td = psum.tile([4, P], fp, tag="ps_bb")
    nc.tensor.transpose(ps_tg[:], s8[:, 0:4], ident[:])
    nc.tensor.transpose(ps_td[:], s8[:, 4:8], ident[:])
    t8g = pool.tile([4, P], fp)
    t8d = pool.tile([4, P], fp)
    V.tensor_copy(t8g[:], ps_tg[:])
    V.tensor_copy(t8d[:], ps_td[:])
    ch = pool.tile([4, P], fp)
    V.tensor_tensor_scan(ch[:], t8g[:], t8d[:], 0.0, OP.mult, OP.add)
    chs = pool.tile([4, P], fp)   # exclusive: shift right by one, col0 = 0
    V.memset(chs[:, 0:1], 0.0)
    V.tensor_copy(chs[:, 1:P], ch[:, 0:P - 1])
    ps_c = psum.tile([P, 4], fp, tag="ps_cc")
    nc.tensor.matmul(ps_c[:], chs[:], ident8[0:4, 0:4])
    dIn = pool.tile([P, 4], fp)
    V.tensor_copy(dIn[:], ps_c[:])

    # ---------- corrections: d = d0 + G * dIn (write into logits planes) ----------
    d = [logits[:, c * W:(c + 1) * W] for c in range(3)]
    l = pool.tile([P, W], fp)
    V.scalar_tensor_tensor(l[:], Gp[:], dIn[:, 3:4], l0[:], OP.mult, OP.add)
    for c in range(3):
        V.scalar_tensor_tensor(d[c], Gp[:], dIn[:, c:c + 1], d0[c][:],
                               OP.mult, OP.add)

    # ---------- run means at run-ends; reverse hold-scan broadcast ----------
    rl = d0[1]
    _act_recip(nc, rl[:], l[:])  # table recip; error only reaches run means
    ie = pool.tile([P, W], fp)
    A.activation(ie[:], e, AF.Identity, scale=-1.0,
                 bias=nc.const_aps.tensor(1.0, (P, 1)))  # 1-e
    h = l0  # dead after l
    V.tensor_tensor(h[:], ie[:], rl[:], OP.mult)
    dat = [pool.tile([P, W], fp, name=f"dat_{c}") for c in range(3)]
    for c in range(3):
        V.tensor_tensor(dat[c][:], d[c][:], h[:], OP.mult)
    m0 = [pool.tile([P, W], fp, name=f"m0_{c}") for c in range(3)]
    for c in range(3):
        V.tensor_tensor_scan(m0[c][:, ::-1], e[:, ::-1], dat[c][:, ::-1],
                             0.0, OP.mult, OP.add)
    # reverse chain across partitions (descending p)
    Erev = pool.tile([P, W], fp)
    V.tensor_tensor(Erev[:], seg[:], seg[:, W - 1:W].to_broadcast([P, W]), OP.is_equal)
    V.tensor_scalar_mul(Erev[:], Erev[:], cmp[:, W:W + 1])
    s8r = pool.tile([P, 8], fp)
    V.tensor_copy(s8r[:, 0:4], Erev[:, 0:1].to_broadcast([P, 4]))
    for c in range(3):
        V.tensor_copy(s8r[:, 4 + c:5 + c], m0[c][:, 0:1])
    V.memset(s8r[:, 7:8], 0.0)
    ps_t2g = psum.tile([4, P], fp, tag="ps_a")
    ps_t2d = psum.tile([4, P], fp, tag="ps_bb")
    nc.tensor.transpose(ps_t2g[:], s8r[:, 0:4], ident[:])
    nc.tensor.transpose(ps_t2d[:], s8r[:, 4:8], ident[:])
    t8rg = pool.tile([4, P], fp)
    t8rd = pool.tile([4, P], fp)
    V.tensor_copy(t8rg[:], ps_t2g[:])
    V.tensor_copy(t8rd[:], ps_t2d[:])
    chr_ = pool.tile([4, P], fp)
    V.tensor_tensor_scan(chr_[:, ::-1], t8rg[:, ::-1], t8rd[:, ::-1],
                         0.0, OP.mult, OP.add)
    chrs = pool.tile([4, P], fp)  # mIn[p] = chr_[p+1], col W-1... col P-1 = 0
    V.memset(chrs[:, P - 1:P], 0.0)
    V.tensor_copy(chrs[:, 0:P - 1], chr_[:, 1:P])
    ps_c2 = psum.tile([P, 4], fp, tag="ps_cc")
    nc.tensor.matmul(ps_c2[:], chrs[:], ident8[0:4, 0:4])
    mIn = pool.tile([P, 4], fp)
    V.tensor_copy(mIn[:], ps_c2[:])
    # means, broadcast over runs: m = m0 + Erev * mIn (into logits planes)
    m = d  # logits planes; d dead after dat
    for c in (0, 1, 2):
        V.scalar_tensor_tensor(m[c], Erev[:], mIn[:, c:c + 1], m0[c][:],
                               OP.mult, OP.add)

    tap("m0c", m[0])
    tap("m1c", m[1])
    tap("m2c", m[2])
    # ---------- coefficients (ACT chain overlaps DVE FIR work) ----------
    bias_w = pool.tile([P, 1], fp)
    V.memset(bias_w[:], LOG_MIN_W)
    bias_q = pool.tile([P, 1], fp)
    V.memset(bias_q[:], -LOG_MIN_Q)
    bias_hp = pool.tile([P, 1], fp)
    V.memset(bias_hp[:], float(np.pi / 2))
    sg = dat  # reuse: dat tiles dead after the m rescans
    gmin_b = pool.tile([P, 1], fp)
    V.memset(gmin_b[:], GAIN_MIN)
    A.activation(sg[0][:], m[0][:], AF.Sigmoid)
    gain = ie  # h dead after dat
    V.tensor_scalar(gain[:], sg[0][:], GAIN_MAX - GAIN_MIN, GAIN_MIN,
                    OP.mult, OP.add)
    A.activation(sg[1][:], m[1][:], AF.Sigmoid)
    w = d0[1]  # d0 tiles dead after dat computed
    A.activation(w[:], sg[1][:], AF.Exp, bias=bias_w[:],
                 scale=(LOG_MAX_W - LOG_MIN_W))
    A.activation(sg[2][:], m[2][:], AF.Sigmoid)
    qinv = d0[2]
    A.activation(qinv[:], sg[2][:], AF.Exp, bias=bias_q[:],
                 scale=-(LOG_MAX_Q - LOG_MIN_Q))
    sinw = d0[0]
    A.activation(sinw[:], w[:], AF.Sin)
    x = m0[0]  # m0 dead after the m corrections
    V.tensor_tensor(x[:], noise[:], gain[:], OP.mult)
    # ---------- FIR accumulate (unscaled): t = x + 2*x[-1] + x[-2] ----------
    ps_x = psum.tile([P, 2], fp, tag="ps_small")
    nc.tensor.matmul(ps_x[:], sh_up[1][:], x[:, W - 2:W])
    xb = pool.tile([P, 2], fp)   # (x[p-1, W-2], x[p-1, W-1]); row0 = 0
    V.tensor_copy(xb[:], ps_x[:])
    s_f = m0[1]
    V.scalar_tensor_tensor(s_f[:, 2:], x[:, 1:W - 1], 2.0, x[:, 2:], OP.mult, OP.add)
    f = m0[2]
    V.tensor_tensor(f[:, 2:], s_f[:, 2:], x[:, :W - 2], OP.add)
    V.scalar_tensor_tensor(s_f[:, 0:1], xb[:, 1:2], 2.0, x[:, 0:1], OP.mult, OP.add)
    V.tensor_tensor(f[:, 0:1], s_f[:, 0:1], xb[:, 0:1], OP.add)
    V.scalar_tensor_tensor(s_f[:, 1:2], x[:, 0:1], 2.0, x[:, 1:2], OP.mult, OP.add)
    V.tensor_tensor(f[:, 1:2], s_f[:, 1:2], xb[:, 1:2], OP.add)

    # ---------- remaining biquad coefficients ----------
    alpha = Erev  # dead after m corrections
    V.scalar_tensor_tensor(alpha[:], sinw[:], 0.5, qinv[:], OP.mult, OP.mult)
    r0a = m0[1]  # s_f scratch, dead once the FIR sums are built
    _act_recip(nc, r0a[:], alpha[:], bias=1.0)              # ~1/(1+alpha)
    two_b = pool.tile([P, 1], fp)
    V.memset(two_b[:], 2.0)
    half_b = pool.tile([P, 1], fp)
    V.memset(half_b[:], 0.5)
    nsc2 = d0[2]
    V.scalar_tensor_tensor(nsc2[:], alpha[:], 1.0, r0a[:],
                           OP.add, OP.mult)                  # (1+alpha)*r0
    cosw = l0  # dead after rl
    A.activation(cosw[:], w[:], AF.Sin, bias=bias_hp[:], scale=-1.0)
    nsc3 = sg[1]  # dead after w
    A.activation(nsc3[:], nsc2[:], AF.Identity, scale=-1.0, bias=two_b[:])
    inva0 = cmp  # dead after m corrections
    V.tensor_tensor(inva0[:, 0:W], nsc3[:], r0a[:], OP.mult)
    b0pre = sg[2]  # dead after qinv
    A.activation(b0pre[:], cosw[:], AF.Identity, scale=-0.5, bias=half_b[:])
    b0 = pool.tile([P, W], fp)
    V.tensor_tensor(b0[:], b0pre[:], inva0[:, 0:W], OP.mult)
    na1 = pool.tile([P, W], fp)
    V.scalar_tensor_tensor(na1[:], cosw[:], 2.0, inva0[:, 0:W], OP.mult, OP.mult)
    na2 = pool.tile([P, W], fp)
    V.scalar_tensor_tensor(na2[:], alpha[:], 1.0, inva0[:, 0:W], OP.subtract, OP.mult)
    tap("inva0", inva0[:, 0:W])
    tap("b0", b0[:])
    tap("na1", na1[:])
    tap("na2", na2[:])
    fsc = pool.tile([P, W], fp)
    V.tensor_tensor(fsc[:], f[:], b0[:], OP.mult)
    f = fsc

    # ---------- double-step composite coefficients ----------
    # pair m covers steps n=2m, n=2m+1:
    #   v_n     = na1_n v_{n-1} + na2_n v_{n-2} (+ f_n)
    #   v_{n+1} = A_m  v_{n-1} + B_m  v_{n-2} (+ F_m)
    # with A = na1_{n+1} na1_n + na2_{n+1}, B = na1_{n+1} na2_n,
    #      F = na1_{n+1} f_n + f_{n+1}.
    Lh = L // 2
    na13 = na1.rearrange("p (c n) -> p c n", c=C)
    na23 = na2.rearrange("p (c n) -> p c n", c=C)
    f3 = f.rearrange("p (c n) -> p c n", c=C)
    n1e = na13[:, :, 0:L:2]
    n1o = na13[:, :, 1:L:2]
    n2e = na23[:, :, 0:L:2]
    n2o = na23[:, :, 1:L:2]
    Bm = pool.tile([P, C * Lh], fp)
    Bm3 = Bm.rearrange("p (c m) -> p c m", c=C)
    V.tensor_tensor(Bm3[:], n1o, n2e, OP.mult)
    Amt = pool.tile([P, C * Lh], fp)
    Amt3 = Amt.rearrange("p (c m) -> p c m", c=C)
    V.tensor_tensor(Amt3[:], n1o, n1e, OP.mult)
    Am = pool.tile([P, C * Lh], fp)
    Am3 = Am.rearrange("p (c m) -> p c m", c=C)
    V.tensor_tensor(Am3[:], Amt3[:], n2o, OP.add)
    fD = pool.tile([P, C * Lh * 2], fp)
    fD4 = fD.rearrange("p (c m k) -> p c m k", c=C, m=Lh, k=2)
    V.tensor_tensor(fD4[:, :, :, 1:2], n1o.unsqueeze(3), f3[:, :, 0:L:2].unsqueeze(3),
                    OP.mult)
    V.tensor_tensor(fD4[:, :, :, 1:2], fD4[:, :, :, 1:2], f3[:, :, 1:L:2].unsqueeze(3),
                    OP.add)
    V.tensor_copy(fD4[:, :, :, 0:1], f3[:, :, 0:L:2].unsqueeze(3))
    coefD = pool.tile([P, C * Lh * 12], fp)
    cD4 = coefD.rearrange("p (c m k) -> p c m k", c=C, m=Lh, k=12)
    A.activation(cD4[:, :, :, 0:3], n2e.unsqueeze(3).to_broadcast([P, C, Lh, 3]),
                 AF.Copy)
    A.activation(cD4[:, :, :, 3:6], n1e.unsqueeze(3).to_broadcast([P, C, Lh, 3]),
                 AF.Copy)
    V.tensor_copy(cD4[:, :, :, 6:9], Bm3.unsqueeze(3).to_broadcast([P, C, Lh, 3]))
    V.tensor_copy(cD4[:, :, :, 9:12], Am3.unsqueeze(3).to_broadcast([P, C, Lh, 3]))

    # ---------- within-chunk recursions (y_zs, p, q interleaved) ----------
    # ypq[P, C, (L+2)*3]: slot k holds 3 values (y, p, q) for recursion index
    # k-2; slots 0,1 are the initial conditions.
    ypq = pool.tile([P, C * (L + 2) * 3], fp)
    ypq3 = ypq.rearrange("p (c m) -> p c m", c=C)
    V.memset(ypq3[:, :, 0:6], 0.0)
    V.memset(ypq3[:, :, 2:3], 1.0)   # q_{-2} = 1
    V.memset(ypq3[:, :, 4:5], 1.0)   # p_{-1} = 1
    u = pool.tile([P, C * 12], fp)
    u4 = u.rearrange("p (c s k) -> p c s k", c=C, s=2, k=6)
    for m in range(Lh):
        n = 2 * m
        prevs = ypq3[:, :, 3 * n:3 * n + 6].unsqueeze(2).to_broadcast(
            [P, C, 2, 6])
        coefv = cD4[:, :, m, :].rearrange("p c (s k) -> p c s k", s=2, k=6)
        V.tensor_tensor(u4[:], prevs, coefv, OP.mult)
        V.tensor_tensor(
            ypq3[:, :, 3 * n + 6:3 * n + 12].rearrange(
                "p c (s k) -> p c s k", s=2, k=3),
            u4[:, :, :, 0:3], u4[:, :, :, 3:6], OP.add)
        V.tensor_tensor(ypq3[:, :, 3 * n + 6:3 * n + 10:3],
                        ypq3[:, :, 3 * n + 6:3 * n + 10:3],
                        fD4[:, :, m, :], OP.add)

    tap("f", f[:])
    tap("coefD", coefD[:])
    tap("fD", fD[:])
    tap("ypq", ypq[:])
    # ---------- pair-composed chunk maps + 3-basis walk ----------
    # Pair k combines chunks (2k, 2k+1); the leftover chunk C-1 is applied as
    # a final single step.  Pair-map layout: (d2, p2, q2, d1, p1, q1).
    NPAIR = C // 2
    NSTEP = NPAIR + 1
    base = 3 * L
    arow1 = ypq3[:, 0:2 * NPAIR:2, base + 3:base + 6]   # (d1,p1,q1) of evens
    arow2 = ypq3[:, 0:2 * NPAIR:2, base:base + 3]       # (d2,p2,q2) of evens
    mapsP = pool.tile([P, NPAIR * 6], fp)
    mp3 = mapsP.rearrange("p (k m) -> p k m", k=NPAIR)
    vA = pool.tile([P, NPAIR * 3], fp)
    vB = pool.tile([P, NPAIR * 3], fp)
    vC = pool.tile([P, NPAIR * 3], fp)
    v3a = vA.rearrange("p (k m) -> p k m", k=NPAIR)
    v3b = vB.rearrange("p (k m) -> p k m", k=NPAIR)
    v3c = vC.rearrange("p (k m) -> p k m", k=NPAIR)

    def bsc(col):
        return ypq3[:, 1:2 * NPAIR + 1:2, base + col:base + col + 1]

    for (pc, qc, dc), off in (((4, 5, 3), 3), ((1, 2, 0), 0)):
        V.tensor_tensor(v3a[:], arow1, bsc(pc).to_broadcast([P, NPAIR, 3]),
                        OP.mult)
        V.tensor_tensor(v3b[:], arow2, bsc(qc).to_broadcast([P, NPAIR, 3]),
                        OP.mult)
        V.tensor_tensor(v3c[:], v3a[:], v3b[:], OP.add)
        V.tensor_tensor(mp3[:, :, off:off + 1], v3c[:, :, 0:1], bsc(dc), OP.add)
        V.tensor_copy(mp3[:, :, off + 1:off + 3], v3c[:, :, 1:3])

    # walk: slot j holds incoming state of chunk 2j (j < NSTEP); the final
    # slot NSTEP is the partition's outgoing state.
    # state slot pair order: (beta, alpha) = (y_{-2}, y_{-1}); walks: 0 = zero
    # state, 1 = alpha basis, 2 = beta basis.
    S = pool.tile([P, 3 * (NSTEP + 1) * 2], fp)
    S4 = S.rearrange("p (w s k) -> p w s k", w=3, s=NSTEP + 1, k=2)
    V.memset(S[:], 0.0)
    V.memset(S4[:, 1:2, 0:1, 1:2], 1.0)
    V.memset(S4[:, 2:3, 0:1, 0:1], 1.0)
    wk = pool.tile([P, 12], fp)
    wk4 = wk.rearrange("p (w r s) -> p w r s", w=3, r=2, s=2)
    wkb = pool.tile([P, 6], fp)
    wkb3 = wkb.rearrange("p (w r) -> p w r", w=3, r=2)
    for j in range(NSTEP):
        if j < NPAIR:
            bv2 = mp3[:, j, :].rearrange("p (a b) -> p a b", a=2, b=3)
        else:
            c = 2 * NPAIR
            bv2 = ypq3[:, c, base:base + 6].rearrange("p (a b) -> p a b",
                                                      a=2, b=3)
        W4 = bv2[:, :, 1:3].unsqueeze(1).to_broadcast([P, 3, 2, 2])
        dpv = bv2[:, :, 0:1].unsqueeze(1).to_broadcast([P, 3, 2, 1]).rearrange(
            "p w r s -> p w (r s)")
        # (alpha, beta) repeated per row: stored order is (beta, alpha)
        X = S4[:, :, j:j + 1, ::-1].rearrange(
            "p w s k -> p w (s k)").unsqueeze(2).to_broadcast([P, 3, 2, 2])
        V.tensor_tensor(wk4[:], W4, X, OP.mult)
        V.tensor_tensor(wkb3[:], wk4[:, :, :, 0:1].rearrange(
            "p w r s -> p w (r s)"), wk4[:, :, :, 1:2].rearrange(
            "p w r s -> p w (r s)"), OP.add)
        V.tensor_tensor(S4[:, :, j + 1, :], wkb3[:], dpv, OP.add)

    # ---------- partition-level affine maps ----------
    # Mcur[P, 6] = (d1, p1, q1, d2, p2, q2):  alpha' = p1 a + q1 b + d1 etc.
    Mcur = pool.tile([P, 6], fp)
    Snap = S4[:, :, NSTEP:NSTEP + 1, :]  # [P, 3, 1, 2]
    for row, comp in ((0, 1), (1, 0)):  # row 0: alpha (k=1), row 1: beta (k=0)
        sv = Snap[:, :, :, comp:comp + 1].rearrange("p a b c -> p (a b c)")
        dsc = Snap[:, 0:1, :, comp:comp + 1].rearrange(
            "p a b c -> p (a b c)").to_broadcast([P, 3])
        V.tensor_tensor(Mcur[:, 3 * row:3 * row + 3], sv, dsc, OP.subtract)
        V.tensor_copy(Mcur[:, 3 * row:3 * row + 1],
                      Snap[:, 0:1, :, comp:comp + 1].rearrange(
                          "p a b c -> p (a b c)"))

    # ---------- Hillis-Steele inclusive scan of affine maps over partitions ----
    Mnew = pool.tile([P, 6], fp)
    ash = pool.tile([P, 6], fp)
    v6 = pool.tile([P, 6], fp)
    u1t = pool.tile([P, 12], fp)
    u2t = pool.tile([P, 6], fp)
    ps_m = psum.tile([P, 6], fp)
    idmap = pool.tile([P, 6], fp)
    V.memset(idmap[:], 0.0)
    V.memset(idmap[:, 1:2], 1.0)
    V.memset(idmap[:, 5:6], 1.0)
    cur, new = Mcur, Mnew
    for s in (1, 2, 4, 8, 16, 32, 64):
        nc.tensor.matmul(ps_m[:], sh_up[s][:], cur[:])
        V.tensor_tensor(ash[:], ps_m[:], idpad[s][:], OP.add)
        bd = cur[:, 0:4:3].unsqueeze(2)                    # [P, 2, 1]
        # fused: u[r, t, k] = a_group[t][k] * b_scalar[r][t]
        a4 = ash.rearrange("p (t k) -> p t k", t=2).unsqueeze(1).to_broadcast(
            [P, 2, 2, 3])
        b4 = cur.rearrange("p (r k) -> p r k", r=2)[:, :, 1:3].unsqueeze(
            3).to_broadcast([P, 2, 2, 3])
        u1 = u1t.rearrange("p (r t k) -> p r t k", r=2, t=2)
        v = v6.rearrange("p (r k) -> p r k", r=2)
        nw = new.rearrange("p (r k) -> p r k", r=2)
        V.tensor_tensor(u1[:], a4, b4, OP.mult)
        V.tensor_tensor(v[:], u1[:, :, 0, :], u1[:, :, 1, :], OP.add)
        V.tensor_tensor(nw[:, :, 0:1], v[:, :, 0:1], bd, OP.add)
        V.tensor_copy(nw[:, :, 1:3], v[:, :, 1:3])
        cur, new = new, cur
    # exclusive d: alpha0/beta0 per partition = d-cols of T^hat_{p-1}
    ps_d = psum.tile([P, 2], fp, tag="ps_small")
    nc.tensor.matmul(ps_d[:], sh_up[1][:], cur[:, 0:4:3])
    ab0 = pool.tile([P, 2], fp)   # (alpha0, beta0)
    V.tensor_copy(ab0[:], ps_d[:])

    # ---------- true per-chunk incoming states ----------
    # even slots: s_true = s_w0 + alpha0*(s_w1-s_w0) + beta0*(s_w2-s_w0);
    # odd chunks: apply the even chunk's own map to the true even state.
    dl2 = pool.tile([P, 2 * NSTEP], fp)
    dl3 = pool.tile([P, 2 * NSTEP], fp)
    tre = pool.tile([P, 2 * NSTEP], fp)  # [alpha_true 0:NSTEP | beta_true]
    for comp, off in ((1, 0), (0, NSTEP)):
        s0 = S4[:, 0:1, 0:NSTEP, comp:comp + 1].rearrange("p a b c -> p (a b c)")
        s1 = S4[:, 1:2, 0:NSTEP, comp:comp + 1].rearrange("p a b c -> p (a b c)")
        s2 = S4[:, 2:3, 0:NSTEP, comp:comp + 1].rearrange("p a b c -> p (a b c)")
        V.tensor_tensor(dl2[:, off:off + NSTEP], s1, s0, OP.subtract)
        V.tensor_tensor(dl3[:, off:off + NSTEP], s2, s0, OP.subtract)
        V.scalar_tensor_tensor(tre[:, off:off + NSTEP], dl2[:, off:off + NSTEP],
                               ab0[:, 0:1], s0, OP.mult, OP.add)
        V.scalar_tensor_tensor(tre[:, off:off + NSTEP], dl3[:, off:off + NSTEP],
                               ab0[:, 1:2], tre[:, off:off + NSTEP],
                               OP.mult, OP.add)
    ale = tre[:, 0:NPAIR]
    bee = tre[:, NSTEP:NSTEP + NPAIR]

    def ev(col):
        return ypq3[:, 0:2 * NPAIR:2, base + col:base + col + 1].rearrange(
            "p c k -> p (c k)")

    od1 = vA[:, 0:NPAIR]
    od2 = vB[:, 0:NPAIR]
    alo = vA[:, NPAIR:2 * NPAIR]
    beo = vB[:, NPAIR:2 * NPAIR]
    V.tensor_tensor(od1[:], ev(4), ale, OP.mult)
    V.tensor_tensor(od2[:], ev(5), bee, OP.mult)
    V.tensor_tensor(od1[:], od1[:], od2[:], OP.add)
    V.tensor_tensor(alo[:], od1[:], ev(3), OP.add)
    V.tensor_tensor(od1[:], ev(1), ale, OP.mult)
    V.tensor_tensor(od2[:], ev(2), bee, OP.mult)
    V.tensor_tensor(od1[:], od1[:], od2[:], OP.add)
    V.tensor_tensor(beo[:], od1[:], ev(0), OP.add)
    atr = pool.tile([P, 2 * C], fp)   # cols: [alpha_true (C), beta_true (C)]
    V.tensor_copy(atr[:, 0:C:2], tre[:, 0:NSTEP])
    V.tensor_copy(atr[:, 1:C:2], alo[:])
    V.tensor_copy(atr[:, C:2 * C:2], tre[:, NSTEP:2 * NSTEP])
    V.tensor_copy(atr[:, C + 1:2 * C:2], beo[:])

    # ---------- correction pass: y = y_zs + p*alpha_c + q*beta_c ----------
    yfin = pool.tile([P, W], fp)
    y3 = yfin.rearrange("p (c n) -> p c n", c=C)
    t1 = pool.tile([P, W], fp)
    t13 = t1.rearrange("p (c n) -> p c n", c=C)
    t2 = pool.tile([P, W], fp)
    t23 = t2.rearrange("p (c n) -> p c n", c=C)
    alc = atr[:, 0:C].unsqueeze(2).to_broadcast([P, C, L])
    bec = atr[:, C:2 * C].unsqueeze(2).to_broadcast([P, C, L])
    Ch = C // 2
    for lo, hi in ((0, Ch), (Ch, C)):
        pv = ypq3[:, lo:hi, 7:6 + 3 * L:3]
        qv = ypq3[:, lo:hi, 8:6 + 3 * L:3]
        yzs = ypq3[:, lo:hi, 6:4 + 3 * L:3]
        V.tensor_tensor(t13[:, lo:hi, :], pv, alc[:, lo:hi, :], OP.mult)
        V.tensor_tensor(t23[:, lo:hi, :], qv, bec[:, lo:hi, :], OP.mult)
        V.tensor_tensor(y3[:, lo:hi, :], t13[:, lo:hi, :], yzs, OP.add)
        V.tensor_tensor(y3[:, lo:hi, :], y3[:, lo:hi, :], t23[:, lo:hi, :],
                        OP.add)
        nc.sync.dma_start(d_y[:, lo * L:hi * L], yfin[:, lo * L:hi * L])
    tap("atr", atr[:])


_NC_CACHE = None


def _get_nc():
    global _NC_CACHE
    if _NC_CACHE is None:
        _NC_CACHE = build_program()
    return _NC_CACHE


def make_in_maps(noise, seg, lg):
    maps = []
    for r in range(len(noise)):
        s2 = seg[r].reshape(P, W)
        bnd = np.zeros((P, 2), np.float32)
        bnd[1:, 0] = (s2[1:, 0] == s2[:-1, W - 1])
        bnd[:-1, 1] = (s2[1:, 0] == s2[:-1, W - 1])
        maps.append({
            "noise": noise[r].reshape(P, W),
            "seg": s2,
            "logits": np.concatenate(
                [lg[r, :, c].reshape(P, W) for c in range(3)], axis=1),
            "bnd": bnd,
        })
    return maps


def kernel(noise_bursts, segment_ids, logits):
    from concourse.bass_utils import run_bass_kernel_spmd

    noise = np.ascontiguousarray(np.asarray(noise_bursts, dtype=np.float32))
    seg = np.ascontiguousarray(np.asarray(segment_ids).astype(np.int32))
    lg = np.ascontiguousarray(np.asarray(logits, dtype=np.float32))
    assert noise.shape == (B, T) and seg.shape == (B, T) and lg.shape == (B, T, 3)

    nc = _get_nc()
    in_maps = make_in_maps(noise, seg, lg)
    res = run_bass_kernel_spmd(nc, in_maps, list(range(B)))
    out = np.stack([res.results[r]["y"].reshape(T) for r in range(B)])
    return out.astype(np.float32)


# revision 42
# speedup vs baseline: 1.0240x; 1.0060x over previous
"""Trainium2 Bass kernel for nn_DynamicsShaper: time-varying RBJ lowpass biquad
driven by per-segment-averaged logits.

Sharding: batch row r -> NeuronCore r (8 rows, 8 cores, fully independent).

Per-core layout: the row of T=160000 samples is viewed as [128 partitions x
W=1250].  First-order recurrences (segmented cumsum for run means, reverse
hold-scan for broadcast) use the DVE TensorTensorScan instruction per
partition, chained across partitions via a PE transpose + a [.,128] scan.
The order-2 IIR uses a blocked scan: C=25 chunks of L=50 per partition run
three coupled recursions (zero-state response + two homogeneous solutions)
in lockstep, then chunk-to-chunk affine state maps are combined by a
3-basis walk within each partition and a log2(128)-round Hillis-Steele
(PE shift matrices) across partitions, followed by a linear correction pass.
"""

import sys

sys.path.insert(0, "/opt/trn_rl_repo")

import numpy as np

import concourse.bass as bass
import concourse.bacc as bacc
import concourse.mybir as mybir
import concourse.tile as tile
from concourse import masks

P = 128          # SBUF partitions
W = 1250         # samples per partition (T = P*W)
C = 25           # chunks per partition
L = W // C       # chunk length (50)
T = P * W
B = 8
SR = 16000.0
GAIN_MIN, GAIN_MAX = 0.1, 2.0
LOG_MIN_W = float(np.log(2.0 * np.pi * 20.0 / SR))
LOG_MAX_W = float(np.log(np.pi))
LOG_MIN_Q, LOG_MAX_Q = float(np.log(0.0707)), float(np.log(2.0))

fp = mybir.dt.float32
i32 = mybir.dt.int32
OP = mybir.AluOpType
AF = mybir.ActivationFunctionType


def _act_recip(nc, out, in_, bias=0.0, scale=1.0):
    """ACT-table reciprocal 1/(scale*x + bias); refine with Newton after.
    (bass's activation() helper refuses Reciprocal; build the instruction
    directly -- we always follow with a Newton step on DVE.)"""
    eng = nc.scalar
    inputs = [
        eng.lower_ap(in_),
        mybir.ImmediateValue(dtype=mybir.dt.float32, value=float(bias)),
        mybir.ImmediateValue(dtype=mybir.dt.float32, value=float(scale)),
        mybir.ImmediateValue(dtype=mybir.dt.float32, value=0.0),
    ]
    return eng.add_instruction(
        mybir.InstActivation(
            name=nc.get_next_instruction_name(),
            func=AF.Reciprocal,
            ins=inputs,
            outs=[eng.lower_ap(out)],
        )
    )


DEBUG_TAPS = False


def build_program():
    nc = bacc.Bacc("TRN2", target_bir_lowering=False, debug=False, num_devices=B)
    d_noise = nc.dram_tensor("noise", [P, W], fp, kind="ExternalInput").ap()
    d_seg = nc.dram_tensor("seg", [P, W], i32, kind="ExternalInput").ap()
    d_logits = nc.dram_tensor("logits", [P, 3 * W], fp, kind="ExternalInput").ap()
    d_bnd = nc.dram_tensor("bnd", [P, 2], fp, kind="ExternalInput").ap()
    d_y = nc.dram_tensor("y", [P, W], fp, kind="ExternalOutput").ap()
    taps = {}
    if DEBUG_TAPS:
        def tap(name, ap):
            t = nc.dram_tensor(f"dbg_{name}", list(ap.shape), ap.dtype,
                               kind="ExternalOutput").ap()
            nc.sync.dma_start(t, ap)
            taps[name] = t
    else:
        def tap(name, ap):
            pass
    with tile.TileContext(nc) as tc:
        _body(nc, tc, d_noise, d_seg, d_logits, d_bnd, d_y, tap)
    nc.compile()
    return nc


def _body(nc, tc, d_noise, d_seg, d_logits, d_bnd, d_y, tap=lambda n, a: None):
    from contextlib import ExitStack
    ctx = ExitStack()
    pool = ctx.enter_context(tc.tile_pool(name="main", bufs=1))
    psum = ctx.enter_context(tc.tile_pool(name="ps", bufs=1, space="PSUM"))

    V = nc.vector
    G = nc.gpsimd
    A = nc.scalar

    # ---------- loads ----------
    seg = pool.tile([P, W], i32)
    logits = pool.tile([P, 3 * W], fp)
    noise = pool.tile([P, W], fp)
    cmp = pool.tile([P, W + 1], fp)
    nc.sync.dma_start(cmp[:, 0:1], d_bnd[:, 0:1])
    nc.sync.dma_start(cmp[:, W:W + 1], d_bnd[:, 1:2])
    nc.sync.dma_start(seg[:, 0:W // 2], d_seg[:, 0:W // 2])
    nc.sync.dma_start(seg[:, W // 2:W], d_seg[:, W // 2:W])
    for c in range(3):
        nc.sync.dma_start(logits[:, c * W:(c + 1) * W],
                          d_logits[:, c * W:(c + 1) * W])
    nc.sync.dma_start(noise[:], d_noise)

    # ---------- constants: identity + shift matrices ----------
    ident = pool.tile([P, P], fp)
    masks.make_identity(nc, ident[:])
    ident8 = pool.tile([8, 8], fp)
    masks.make_identity(nc, ident8[:])

    zmat = pool.tile([P, P], fp)
    G.memset(zmat[:], 0.0)

    def shift_mat(base):
        m = pool.tile([P, P], fp, name=f"shift_{base}")
        G.affine_select(out=m[:], in_=zmat[:], compare_op=OP.not_equal, fill=1.0,
                        base=base, pattern=[[-1, P]], channel_multiplier=1)
        return m

    sh_up = {s: shift_mat(s) for s in (1, 2, 4, 8, 16, 32, 64)}  # out[p] = in[p-s]

    # identity-affine pads for HS rounds: rows < s get identity map
    # map layout per 6 cols: (d1, p1, q1, d2, p2, q2); identity: p1=1, q2=1
    idpad = {}
    for s in (1, 2, 4, 8, 16, 32, 64):
        t = pool.tile([P, 6], fp, name=f"idpad_{s}")
        V.memset(t[:], 0.0)
        V.memset(t[0:s, 1:2], 1.0)
        V.memset(t[0:s, 5:6], 1.0)
        idpad[s] = t



    # ---------- gates ----------
    # cmp[P, W+1]: cmp[:, j] (1<=j<=W-1) = (seg[j] == seg[j-1]); col 0 = gate
    # at partition start; col W = "continues into next partition".  The two
    # boundary columns are host-computed (d_bnd) since they need cross-
    # partition neighbors.
    V.tensor_tensor(cmp[:, 1:W], seg[:, 1:], seg[:, :W - 1], OP.is_equal)
    g = cmp[:, 0:W]
    e = cmp[:, 1:W + 1]

    # ---------- forward segmented scans (zero init) ----------
    czero = nc.const_aps.tensor(0.0, (P, W))
    cone = nc.const_aps.tensor(1.0, (P, W))
    d0 = [pool.tile([P, W], fp, name=f"d0_{c}") for c in range(3)]
    l0 = pool.tile([P, W], fp)
    Gp = pool.tile([P, W], fp)   # prefix product of gates (ids sorted!)
    V.tensor_tensor(Gp[:], seg[:], seg[:, 0:1].to_broadcast([P, W]), OP.is_equal)
    V.tensor_scalar_mul(Gp[:], Gp[:], cmp[:, 0:1])
    V.tensor_tensor_scan(l0[:], g, cone, 0.0, OP.mult, OP.add)
    for c in range(3):
        V.tensor_tensor_scan(d0[c][:], g, logits[:, c * W:(c + 1) * W],
                             0.0, OP.mult, OP.add)

    # ---------- cross-partition chain for forward scans ----------
    # summaries [P, 8]: (gP, gP, gP, gP, d0_0[W-1], d0_1[W-1], d0_2[W-1], l0[W-1])
    s8 = pool.tile([P, 8], fp)
    V.tensor_copy(s8[:, 0:4], Gp[:, W - 1:W].to_broadcast([P, 4]))
    for c in range(3):
        V.tensor_copy(s8[:, 4 + c:5 + c], d0[c][:, W - 1:W])
    V.tensor_copy(s8[:, 7:8], l0[:, W - 1:W])
    ps_tg = psum.tile([4, P], fp, tag="ps_a")
    ps_td = psum.tile([4, P], fp, tag="ps_bb")
    nc.tensor.transpose(ps_tg[:], s8[:, 0:4], ident[:])
    nc.tensor.transpose(ps_td[:], s8[:, 4:8], ident[:])
    t8g = pool.tile([4, P], fp)
    t8d = pool.tile([4, P], fp)
    V.tensor_copy(t8g[:], ps_tg[:])
    V.tensor_copy(t8d[:], ps_td[:])
    ch = pool.tile([4, P], fp)
    V.tensor_tensor_scan(ch[:], t8g[:], t8d[:], 0.0, OP.mult, OP.add)
    chs = pool.tile([4, P], fp)   # exclusive: shift right by one, col0 = 0
    V.memset(chs[:, 0:1], 0.0)
    V.tensor_copy(chs[:, 1:P], ch[:, 0:P - 1])
    ps_c = psum.tile([P, 4], fp, tag="ps_cc")
    nc.tensor.matmul(ps_c[:], chs[:], ident8[0:4, 0:4])
    dIn = pool.tile([P, 4], fp)
    V.tensor_copy(dIn[:], ps_c[:])

    # ---------- corrections: d = d0 + G * dIn (write into logits planes) ----------
    d = [logits[:, c * W:(c + 1) * W] for c in range(3)]
    l = pool.tile([P, W], fp)
    V.scalar_tensor_tensor(l[:], Gp[:], dIn[:, 3:4], l0[:], OP.mult, OP.add)
    for c in range(3):
        V.scalar_tensor_tensor(d[c], Gp[:], dIn[:, c:c + 1], d0[c][:],
                               OP.mult, OP.add)

    # ---------- run means at run-ends; reverse hold-scan broadcast ----------
    rl = d0[1]
    _act_recip(nc, rl[:], l[:])  # table recip; error only reaches run means
    ie = pool.tile([P, W], fp)
    A.activation(ie[:], e, AF.Identity, scale=-1.0,
                 bias=nc.const_aps.tensor(1.0, (P, 1)))  # 1-e
    h = l0  # dead after l
    V.tensor_tensor(h[:], ie[:], rl[:], OP.mult)
    dat = [pool.tile([P, W], fp, name=f"dat_{c}") for c in range(3)]
    for c in range(3):
        V.tensor_tensor(dat[c][:], d[c][:], h[:], OP.mult)
    m0 = [pool.tile([P, W], fp, name=f"m0_{c}") for c in range(3)]
    for c in range(3):
        V.tensor_tensor_scan(m0[c][:, ::-1], e[:, ::-1], dat[c][:, ::-1],
                             0.0, OP.mult, OP.add)
    # reverse chain across partitions (descending p)
    Erev = pool.tile([P, W], fp)
    V.tensor_tensor(Erev[:], seg[:], seg[:, W - 1:W].to_broadcast([P, W]), OP.is_equal)
    V.tensor_scalar_mul(Erev[:], Erev[:], cmp[:, W:W + 1])
    s8r = pool.tile([P, 8], fp)
    V.tensor_copy(s8r[:, 0:4], Erev[:, 0:1].to_broadcast([P, 4]))
    for c in range(3):
        V.tensor_copy(s8r[:, 4 + c:5 + c], m0[c][:, 0:1])
    V.memset(s8r[:, 7:8], 0.0)
    ps_t2g = psum.tile([4, P], fp, tag="ps_a")
    ps_t2d = psum.tile([4, P], fp, tag="ps_bb")
    nc.tensor.transpose(ps_t2g[:], s8r[:, 0:4], ident[:])
    nc.tensor.transpose(ps_t2d[:], s8r[:, 4:8], ident[:])
    t8rg = pool.tile([4, P], fp)
    t8rd = pool.tile([4, P], fp)
    V.tensor_copy(t8rg[:], ps_t2g[:])
    V.tensor_copy(t8rd[:], ps_t2d[:])
    chr_ = pool.tile([4, P], fp)
    V.tensor_tensor_scan(chr_[:, ::-1], t8rg[:, ::-1], t8rd[:, ::-1],
                         0.0, OP.mult, OP.add)
    chrs = pool.tile([4, P], fp)  # mIn[p] = chr_[p+1], col W-1... col P-1 = 0
    V.memset(chrs[:, P - 1:P], 0.0)
    V.tensor_copy(chrs[:, 0:P - 1], chr_[:, 1:P])
    ps_c2 = psum.tile([P, 4], fp, tag="ps_cc")
    nc.tensor.matmul(ps_c2[:], chrs[:], ident8[0:4, 0:4])
    mIn = pool.tile([P, 4], fp)
    V.tensor_copy(mIn[:], ps_c2[:])
    # means, broadcast over runs: m = m0 + Erev * mIn (into logits planes)
    m = d  # logits planes; d dead after dat
    for c in (0, 1, 2):
        V.scalar_tensor_tensor(m[c], Erev[:], mIn[:, c:c + 1], m0[c][:],
                               OP.mult, OP.add)

    tap("m0c", m[0])
    tap("m1c", m[1])
    tap("m2c", m[2])
    # ---------- coefficients (ACT chain overlaps DVE FIR work) ----------
    bias_w = pool.tile([P, 1], fp)
    V.memset(bias_w[:], LOG_MIN_W)
    bias_q = pool.tile([P, 1], fp)
    V.memset(bias_q[:], -LOG_MIN_Q)
    bias_hp = pool.tile([P, 1], fp)
    V.memset(bias_hp[:], float(np.pi / 2))
    sg = dat  # reuse: dat tiles dead after the m rescans
    gmin_b = pool.tile([P, 1], fp)
    V.memset(gmin_b[:], GAIN_MIN)
    A.activation(sg[0][:], m[0][:], AF.Sigmoid)
    gain = ie  # h dead after dat
    V.tensor_scalar(gain[:], sg[0][:], GAIN_MAX - GAIN_MIN, GAIN_MIN,
                    OP.mult, OP.add)
    A.activation(sg[1][:], m[1][:], AF.Sigmoid)
    w = d0[1]  # d0 tiles dead after dat computed
    A.activation(w[:], sg[1][:], AF.Exp, bias=bias_w[:],
                 scale=(LOG_MAX_W - LOG_MIN_W))
    A.activation(sg[2][:], m[2][:], AF.Sigmoid)
    qinv = d0[2]
    A.activation(qinv[:], sg[2][:], AF.Exp, bias=bias_q[:],
                 scale=-(LOG_MAX_Q - LOG_MIN_Q))
    sinw = d0[0]
    A.activation(sinw[:], w[:], AF.Sin)
    cosw = l0  # dead after rl (h/s_f readers long done)
    A.activation(cosw[:], w[:], AF.Sin, bias=bias_hp[:], scale=-1.0)
    x = m0[0]  # m0 dead after the m corrections
    V.tensor_tensor(x[:], noise[:], gain[:], OP.mult)
    # ---------- FIR accumulate (unscaled): t = x + 2*x[-1] + x[-2] ----------
    ps_x = psum.tile([P, 2], fp, tag="ps_small")
    nc.tensor.matmul(ps_x[:], sh_up[1][:], x[:, W - 2:W])
    xb = pool.tile([P, 2], fp)   # (x[p-1, W-2], x[p-1, W-1]); row0 = 0
    V.tensor_copy(xb[:], ps_x[:])
    s_f = m0[1]
    V.scalar_tensor_tensor(s_f[:, 2:], x[:, 1:W - 1], 2.0, x[:, 2:], OP.mult, OP.add)
    f = m0[2]
    V.tensor_tensor(f[:, 2:], s_f[:, 2:], x[:, :W - 2], OP.add)
    V.scalar_tensor_tensor(s_f[:, 0:1], xb[:, 1:2], 2.0, x[:, 0:1], OP.mult, OP.add)
    V.tensor_tensor(f[:, 0:1], s_f[:, 0:1], xb[:, 0:1], OP.add)
    V.scalar_tensor_tensor(s_f[:, 1:2], x[:, 0:1], 2.0, x[:, 1:2], OP.mult, OP.add)
    V.tensor_tensor(f[:, 1:2], s_f[:, 1:2], xb[:, 1:2], OP.add)

    # ---------- remaining biquad coefficients ----------
    alpha = Erev  # dead after m corrections
    V.scalar_tensor_tensor(alpha[:], sinw[:], 0.5, qinv[:], OP.mult, OP.mult)
    r0a = m0[1]  # s_f scratch, dead once the FIR sums are built
    _act_recip(nc, r0a[:], alpha[:], bias=1.0)              # ~1/(1+alpha)
    two_b = pool.tile([P, 1], fp)
    V.memset(two_b[:], 2.0)
    half_b = pool.tile([P, 1], fp)
    V.memset(half_b[:], 0.5)
    nsc2 = d0[2]
    V.scalar_tensor_tensor(nsc2[:], alpha[:], 1.0, r0a[:],
                           OP.add, OP.mult)                  # (1+alpha)*r0
    nsc3 = sg[1]  # dead after w
    A.activation(nsc3[:], nsc2[:], AF.Identity, scale=-1.0, bias=two_b[:])
    inva0 = cmp  # dead after m corrections
    V.tensor_tensor(inva0[:, 0:W], nsc3[:], r0a[:], OP.mult)
    b0pre = sg[2]  # dead after qinv
    A.activation(b0pre[:], cosw[:], AF.Identity, scale=-0.5, bias=half_b[:])
    b0 = pool.tile([P, W], fp)
    V.tensor_tensor(b0[:], b0pre[:], inva0[:, 0:W], OP.mult)
    na1 = pool.tile([P, W], fp)
    V.scalar_tensor_tensor(na1[:], cosw[:], 2.0, inva0[:, 0:W], OP.mult, OP.mult)
    na2 = pool.tile([P, W], fp)
    V.scalar_tensor_tensor(na2[:], alpha[:], 1.0, inva0[:, 0:W], OP.subtract, OP.mult)
    tap("inva0", inva0[:, 0:W])
    tap("b0", b0[:])
    tap("na1", na1[:])
    tap("na2", na2[:])
    fsc = pool.tile([P, W], fp)
    V.tensor_tensor(fsc[:], f[:], b0[:], OP.mult)
    f = fsc

    # ---------- double-step composite coefficients ----------
    # pair m covers steps n=2m, n=2m+1:
    #   v_n     = na1_n v_{n-1} + na2_n v_{n-2} (+ f_n)
    #   v_{n+1} = A_m  v_{n-1} + B_m  v_{n-2} (+ F_m)
    # with A = na1_{n+1} na1_n + na2_{n+1}, B = na1_{n+1} na2_n,
    #      F = na1_{n+1} f_n + f_{n+1}.
    Lh = L // 2
    na13 = na1.rearrange("p (c n) -> p c n", c=C)
    na23 = na2.rearrange("p (c n) -> p c n", c=C)
    f3 = f.rearrange("p (c n) -> p c n", c=C)
    n1e = na13[:, :, 0:L:2]
    n1o = na13[:, :, 1:L:2]
    n2e = na23[:, :, 0:L:2]
    n2o = na23[:, :, 1:L:2]
    Bm = pool.tile([P, C * Lh], fp)
    Bm3 = Bm.rearrange("p (c m) -> p c m", c=C)
    V.tensor_tensor(Bm3[:], n1o, n2e, OP.mult)
    Amt = pool.tile([P, C * Lh], fp)
    Amt3 = Amt.rearrange("p (c m) -> p c m", c=C)
    V.tensor_tensor(Amt3[:], n1o, n1e, OP.mult)
    Am = pool.tile([P, C * Lh], fp)
    Am3 = Am.rearrange("p (c m) -> p c m", c=C)
    V.tensor_tensor(Am3[:], Amt3[:], n2o, OP.add)
    fD = pool.tile([P, C * Lh * 2], fp)
    fD4 = fD.rearrange("p (c m k) -> p c m k", c=C, m=Lh, k=2)
    V.tensor_tensor(fD4[:, :, :, 1:2], n1o.unsqueeze(3), f3[:, :, 0:L:2].unsqueeze(3),
                    OP.mult)
    V.tensor_tensor(fD4[:, :, :, 1:2], fD4[:, :, :, 1:2], f3[:, :, 1:L:2].unsqueeze(3),
                    OP.add)
    V.tensor_copy(fD4[:, :, :, 0:1], f3[:, :, 0:L:2].unsqueeze(3))
    coefD = pool.tile([P, C * Lh * 12], fp)
    cD4 = coefD.rearrange("p (c m k) -> p c m k", c=C, m=Lh, k=12)
    A.activation(cD4[:, :, :, 0:3], n2e.unsqueeze(3).to_broadcast([P, C, Lh, 3]),
                 AF.Copy)
    A.activation(cD4[:, :, :, 3:6], n1e.unsqueeze(3).to_broadcast([P, C, Lh, 3]),
                 AF.Copy)
    V.tensor_copy(cD4[:, :, :, 6:9], Bm3.unsqueeze(3).to_broadcast([P, C, Lh, 3]))
    V.tensor_copy(cD4[:, :, :, 9:12], Am3.unsqueeze(3).to_broadcast([P, C, Lh, 3]))

    # ---------- within-chunk recursions (y_zs, p, q interleaved) ----------
    # ypq[P, C, (L+2)*3]: slot k holds 3 values (y, p, q) for recursion index
    # k-2; slots 0,1 are the initial conditions.
    ypq = pool.tile([P, C * (L + 2) * 3], fp)
    ypq3 = ypq.rearrange("p (c m) -> p c m", c=C)
    V.memset(ypq3[:, :, 0:6], 0.0)
    V.memset(ypq3[:, :, 2:3], 1.0)   # q_{-2} = 1
    V.memset(ypq3[:, :, 4:5], 1.0)   # p_{-1} = 1
    u = pool.tile([P, C * 12], fp)
    u4 = u.rearrange("p (c s k) -> p c s k", c=C, s=2, k=6)
    for m in range(Lh):
        n = 2 * m
        prevs = ypq3[:, :, 3 * n:3 * n + 6].unsqueeze(2).to_broadcast(
            [P, C, 2, 6])
        coefv = cD4[:, :, m, :].rearrange("p c (s k) -> p c s k", s=2, k=6)
        V.tensor_tensor(u4[:], prevs, coefv, OP.mult)
        V.tensor_tensor(
            ypq3[:, :, 3 * n + 6:3 * n + 12].rearrange(
                "p c (s k) -> p c s k", s=2, k=3),
            u4[:, :, :, 0:3], u4[:, :, :, 3:6], OP.add)
        V.tensor_tensor(ypq3[:, :, 3 * n + 6:3 * n + 10:3],
                        ypq3[:, :, 3 * n + 6:3 * n + 10:3],
                        fD4[:, :, m, :], OP.add)

    tap("f", f[:])
    tap("coefD", coefD[:])
    tap("fD", fD[:])
    tap("ypq", ypq[:])
    # ---------- pair-composed chunk maps + 3-basis walk ----------
    # Pair k combines chunks (2k, 2k+1); the leftover chunk C-1 is applied as
    # a final single step.  Pair-map layout: (d2, p2, q2, d1, p1, q1).
    NPAIR = C // 2
    NSTEP = NPAIR + 1
    base = 3 * L
    arow1 = ypq3[:, 0:2 * NPAIR:2, base + 3:base + 6]   # (d1,p1,q1) of evens
    arow2 = ypq3[:, 0:2 * NPAIR:2, base:base + 3]       # (d2,p2,q2) of evens
    mapsP = pool.tile([P, NPAIR * 6], fp)
    mp3 = mapsP.rearrange("p (k m) -> p k m", k=NPAIR)
    vA = pool.tile([P, NPAIR * 3], fp)
    vB = pool.tile([P, NPAIR * 3], fp)
    vC = pool.tile([P, NPAIR * 3], fp)
    v3a = vA.rearrange("p (k m) -> p k m", k=NPAIR)
    v3b = vB.rearrange("p (k m) -> p k m", k=NPAIR)
    v3c = vC.rearrange("p (k m) -> p k m", k=NPAIR)

    def bsc(col):
        return ypq3[:, 1:2 * NPAIR + 1:2, base + col:base + col + 1]

    for (pc, qc, dc), off in (((4, 5, 3), 3), ((1, 2, 0), 0)):
        V.tensor_tensor(v3a[:], arow1, bsc(pc).to_broadcast([P, NPAIR, 3]),
                        OP.mult)
        V.tensor_tensor(v3b[:], arow2, bsc(qc).to_broadcast([P, NPAIR, 3]),
                        OP.mult)
        V.tensor_tensor(v3c[:], v3a[:], v3b[:], OP.add)
        V.tensor_tensor(mp3[:, :, off:off + 1], v3c[:, :, 0:1], bsc(dc), OP.add)
        V.tensor_copy(mp3[:, :, off + 1:off + 3], v3c[:, :, 1:3])

    # walk: slot j holds incoming state of chunk 2j (j < NSTEP); the final
    # slot NSTEP is the partition's outgoing state.
    # state slot pair order: (beta, alpha) = (y_{-2}, y_{-1}); walks: 0 = zero
    # state, 1 = alpha basis, 2 = beta basis.
    S = pool.tile([P, 3 * (NSTEP + 1) * 2], fp)
    S4 = S.rearrange("p (w s k) -> p w s k", w=3, s=NSTEP + 1, k=2)
    V.memset(S[:], 0.0)
    V.memset(S4[:, 1:2, 0:1, 1:2], 1.0)
    V.memset(S4[:, 2:3, 0:1, 0:1], 1.0)
    wk = pool.tile([P, 12], fp)
    wk4 = wk.rearrange("p (w r s) -> p w r s", w=3, r=2, s=2)
    wkb = pool.tile([P, 6], fp)
    wkb3 = wkb.rearrange("p (w r) -> p w r", w=3, r=2)
    for j in range(NSTEP):
        if j < NPAIR:
            bv2 = mp3[:, j, :].rearrange("p (a b) -> p a b", a=2, b=3)
        else:
            c = 2 * NPAIR
            bv2 = ypq3[:, c, base:base + 6].rearrange("p (a b) -> p a b",
                                                      a=2, b=3)
        W4 = bv2[:, :, 1:3].unsqueeze(1).to_broadcast([P, 3, 2, 2])
        dpv = bv2[:, :, 0:1].unsqueeze(1).to_broadcast([P, 3, 2, 1]).rearrange(
            "p w r s -> p w (r s)")
        # (alpha, beta) repeated per row: stored order is (beta, alpha)
        X = S4[:, :, j:j + 1, ::-1].rearrange(
            "p w s k -> p w (s k)").unsqueeze(2).to_broadcast([P, 3, 2, 2])
        V.tensor_tensor(wk4[:], W4, X, OP.mult)
        V.tensor_tensor(wkb3[:], wk4[:, :, :, 0:1].rearrange(
            "p w r s -> p w (r s)"), wk4[:, :, :, 1:2].rearrange(
            "p w r s -> p w (r s)"), OP.add)
        V.tensor_tensor(S4[:, :, j + 1, :], wkb3[:], dpv, OP.add)

    # ---------- partition-level affine maps ----------
    # Mcur[P, 6] = (d1, p1, q1, d2, p2, q2):  alpha' = p1 a + q1 b + d1 etc.
    Mcur = pool.tile([P, 6], fp)
    Snap = S4[:, :, NSTEP:NSTEP + 1, :]  # [P, 3, 1, 2]
    for row, comp in ((0, 1), (1, 0)):  # row 0: alpha (k=1), row 1: beta (k=0)
        sv = Snap[:, :, :, comp:comp + 1].rearrange("p a b c -> p (a b c)")
        dsc = Snap[:, 0:1, :, comp:comp + 1].rearrange(
            "p a b c -> p (a b c)").to_broadcast([P, 3])
        V.tensor_tensor(Mcur[:, 3 * row:3 * row + 3], sv, dsc, OP.subtract)
        V.tensor_copy(Mcur[:, 3 * row:3 * row + 1],
                      Snap[:, 0:1, :, comp:comp + 1].rearrange(
                          "p a b c -> p (a b c)"))

    # ---------- Hillis-Steele inclusive scan of affine maps over partitions ----
    Mnew = pool.tile([P, 6], fp)
    ash = pool.tile([P, 6], fp)
    v6 = pool.tile([P, 6], fp)
    u1t = pool.tile([P, 12], fp)
    u2t = pool.tile([P, 6], fp)
    ps_m = psum.tile([P, 6], fp)
    idmap = pool.tile([P, 6], fp)
    V.memset(idmap[:], 0.0)
    V.memset(idmap[:, 1:2], 1.0)
    V.memset(idmap[:, 5:6], 1.0)
    cur, new = Mcur, Mnew
    for s in (1, 2, 4, 8, 16, 32, 64):
        nc.tensor.matmul(ps_m[:], sh_up[s][:], cur[:])
        V.tensor_tensor(ash[:], ps_m[:], idpad[s][:], OP.add)
        bd = cur[:, 0:4:3].unsqueeze(2)                    # [P, 2, 1]
        # fused: u[r, t, k] = a_group[t][k] * b_scalar[r][t]
        a4 = ash.rearrange("p (t k) -> p t k", t=2).unsqueeze(1).to_broadcast(
            [P, 2, 2, 3])
        b4 = cur.rearrange("p (r k) -> p r k", r=2)[:, :, 1:3].unsqueeze(
            3).to_broadcast([P, 2, 2, 3])
        u1 = u1t.rearrange("p (r t k) -> p r t k", r=2, t=2)
        v = v6.rearrange("p (r k) -> p r k", r=2)
        nw = new.rearrange("p (r k) -> p r k", r=2)
        V.tensor_tensor(u1[:], a4, b4, OP.mult)
        V.tensor_tensor(v[:], u1[:, :, 0, :], u1[:, :, 1, :], OP.add)
        V.tensor_tensor(nw[:, :, 0:1], v[:, :, 0:1], bd, OP.add)
        V.tensor_copy(nw[:, :, 1:3], v[:, :, 1:3])
        cur, new = new, cur
    # exclusive d: alpha0/beta0 per partition = d-cols of T^hat_{p-1}
    ps_d = psum.tile([P, 2], fp, tag="ps_small")
    nc.tensor.matmul(ps_d[:], sh_up[1][:], cur[:, 0:4:3])
    ab0 = pool.tile([P, 2], fp)   # (alpha0, beta0)
    V.tensor_copy(ab0[:], ps_d[:])

    # ---------- true per-chunk incoming states ----------
    # even slots: s_true = s_w0 + alpha0*(s_w1-s_w0) + beta0*(s_w2-s_w0);
    # odd chunks: apply the even chunk's own map to the true even state.
    dl2 = pool.tile([P, 2 * NSTEP], fp)
    dl3 = pool.tile([P, 2 * NSTEP], fp)
    tre = pool.tile([P, 2 * NSTEP], fp)  # [alpha_true 0:NSTEP | beta_true]
    for comp, off in ((1, 0), (0, NSTEP)):
        s0 = S4[:, 0:1, 0:NSTEP, comp:comp + 1].rearrange("p a b c -> p (a b c)")
        s1 = S4[:, 1:2, 0:NSTEP, comp:comp + 1].rearrange("p a b c -> p (a b c)")
        s2 = S4[:, 2:3, 0:NSTEP, comp:comp + 1].rearrange("p a b c -> p (a b c)")
        V.tensor_tensor(dl2[:, off:off + NSTEP], s1, s0, OP.subtract)
        V.tensor_tensor(dl3[:, off:off + NSTEP], s2, s0, OP.subtract)
        V.scalar_tensor_tensor(tre[:, off:off + NSTEP], dl2[:, off:off + NSTEP],
                               ab0[:, 0:1], s0, OP.mult, OP.add)
        V.scalar_tensor_tensor(tre[:, off:off + NSTEP], dl3[:, off:off + NSTEP],
                               ab0[:, 1:2], tre[:, off:off + NSTEP],
                               OP.mult, OP.add)
    ale = tre[:, 0:NPAIR]
    bee = tre[:, NSTEP:NSTEP + NPAIR]

    def ev(col):
        return ypq3[:, 0:2 * NPAIR:2, base + col:base + col + 1].rearrange(
            "p c k -> p (c k)")

    od1 = vA[:, 0:NPAIR]
    od2 = vB[:, 0:NPAIR]
    alo = vA[:, NPAIR:2 * NPAIR]
    beo = vB[:, NPAIR:2 * NPAIR]
    V.tensor_tensor(od1[:], ev(4), ale, OP.mult)
    V.tensor_tensor(od2[:], ev(5), bee, OP.mult)
    V.tensor_tensor(od1[:], od1[:], od2[:], OP.add)
    V.tensor_tensor(alo[:], od1[:], ev(3), OP.add)
    V.tensor_tensor(od1[:], ev(1), ale, OP.mult)
    V.tensor_tensor(od2[:], ev(2), bee, OP.mult)
    V.tensor_tensor(od1[:], od1[:], od2[:], OP.add)
    V.tensor_tensor(beo[:], od1[:], ev(0), OP.add)
    atr = pool.tile([P, 2 * C], fp)   # cols: [alpha_true (C), beta_true (C)]
    V.tensor_copy(atr[:, 0:C:2], tre[:, 0:NSTEP])
    V.tensor_copy(atr[:, 1:C:2], alo[:])
    V.tensor_copy(atr[:, C:2 * C:2], tre[:, NSTEP:2 * NSTEP])
    V.tensor_copy(atr[:, C + 1:2 * C:2], beo[:])

    # ---------- correction pass: y = y_zs + p*alpha_c + q*beta_c ----------
    yfin = pool.tile([P, W], fp)
    y3 = yfin.rearrange("p (c n) -> p c n", c=C)
    t1 = pool.tile([P, W], fp)
    t13 = t1.rearrange("p (c n) -> p c n", c=C)
    t2 = pool.tile([P, W], fp)
    t23 = t2.rearrange("p (c n) -> p c n", c=C)
    alc = atr[:, 0:C].unsqueeze(2).to_broadcast([P, C, L])
    bec = atr[:, C:2 * C].unsqueeze(2).to_broadcast([P, C, L])
    Ch = C // 2
    for lo, hi in ((0, Ch), (Ch, C)):
        pv = ypq3[:, lo:hi, 7:6 + 3 * L:3]
        qv = ypq3[:, lo:hi, 8:6 + 3 * L:3]
        yzs = ypq3[:, lo:hi, 6:4 + 3 * L:3]
        V.tensor_tensor(t13[:, lo:hi, :], pv, alc[:, lo:hi, :], OP.mult)
        V.tensor_tensor(t23[:, lo:hi, :], qv, bec[:, lo:hi, :], OP.mult)
        V.tensor_tensor(y3[:, lo:hi, :], t13[:, lo:hi, :], yzs, OP.add)
        V.tensor_tensor(y3[:, lo:hi, :], y3[:, lo:hi, :], t23[:, lo:hi, :],
                        OP.add)
        nc.sync.dma_start(d_y[:, lo * L:hi * L], yfin[:, lo * L:hi * L])
    tap("atr", atr[:])


_NC_CACHE = None


def _get_nc():
    global _NC_CACHE
    if _NC_CACHE is None:
        _NC_CACHE = build_program()
    return _NC_CACHE


def make_in_maps(noise, seg, lg):
    maps = []
    for r in range(len(noise)):
        s2 = seg[r].reshape(P, W)
        bnd = np.zeros((P, 2), np.float32)
        bnd[1:, 0] = (s2[1:, 0] == s2[:-1, W - 1])
        bnd[:-1, 1] = (s2[1:, 0] == s2[:-1, W - 1])
        maps.append({
            "noise": noise[r].reshape(P, W),
            "seg": s2,
            "logits": np.concatenate(
                [lg[r, :, c].reshape(P, W) for c in range(3)], axis=1),
            "bnd": bnd,
        })
    return maps


def kernel(noise_bursts, segment_ids, logits):
    from concourse.bass_utils import run_bass_kernel_spmd

    noise = np.ascontiguousarray(np.asarray(noise_bursts, dtype=np.float32))
    seg = np.ascontiguousarray(np.asarray(segment_ids).astype(np.int32))
    lg = np.ascontiguousarray(np.asarray(logits, dtype=np.float32))
    assert noise.shape == (B, T) and seg.shape == (B, T) and lg.shape == (B, T, 3)

    nc = _get_nc()
    in_maps = make_in_maps(noise, seg, lg)
    res = run_bass_kernel_spmd(nc, in_maps, list(range(B)))
    out = np.stack([res.results[r]["y"].reshape(T) for r in range(B)])
    return out.astype(np.float32)
